# revision 46
# baseline (speedup 1.0000x reference)
"""Distributed GAT (GATConv eval + global mean pool + linear) on 8 TRN2 NeuronCores.

Pipeline (shapes hardcoded for nn_GAT_27968827032308):
  Host: renumber nodes -> 8 cores x 100 blocks x 128 slots (degree-balanced deal
    packing); route each edge (incl. self-loops) to the core owning its dst;
    order per-core edges [block, src-chunk], pad runs to 128-slot tiles; build
    int16 gather-index stream plus per-tile dst-onehot scalar streams.
  Phase A (device, per core): records for own nodes: xh = x@W (PE, bf16),
    a_src/a_dst = x@(W@[Asrc|Adst]); 512B record rows
    [xh bf16 x128 | a_src f32 x4 | pad] -> DRAM shard; a_dst kept in a
    persistent SBUF table [128, BLOCKS*4] bf16 (dst side is always local).
  Phase B: AllGather shards -> replicated full record table.
  Phase C (per 5-block superblock): one dma_gather per src-chunk pulls
    25 record tiles; per block: onehot oh[slot,node] and transposed onehot
    ohT[node,slot] built in single batched DVE ops; a_dst delivered to edge
    slots via per-tile PE matmuls ohT^T @ adst_blk (no second gather);
    p = exp(leaky_0.2(a_src+a_dst)) on the Scalar engine; batched 4D msg
    multiply; PE matmul acc[n,0:128] += oh^T @ (p*xh), acc[n,128:132] += oh^T@p
    accumulated in PSUM per node-block; flush via Scalar Lrelu(acc*1/s).
  Pooling: PE matmul with host-built graph-onehot -> partial pooled sums
    [128 graphs, 128]; AllReduce; * (1/count); final linear via PE.

kernel(**inputs): FULL inputs -> FULL [128, 10] float32 output.
"""

import sys

sys.path.insert(0, "/opt/trn_rl_repo")
sys.path.insert(0, "/opt/trn_rl_repo/concourse")

import numpy as np

N = 100000
FIN = 128
H = 4
C = 32
HC = H * C
B = 128
NCLS = 10
NEG_ATT = 0.2
NEG_ACT = 0.01

NCORES = 8
BLOCKS = 100
P = 128
NPC = BLOCKS * P  # 12800 padded node slots per core
T_BC_MIN = 5
NCHUNK = 4
SHARD_ROWS = NPC + 4  # + pad rows; dummy record at SHARD_ROWS-1
CHUNK_ROWS = 2 * SHARD_ROWS  # 25608 < 32768 (int16 gather idx)
TBL_ROWS = NCORES * SHARD_ROWS
DUMMY_REL = SHARD_ROWS - 1  # same rel for every chunk (core 2c's dummy row)
REC_F32 = 64  # 256B records: [xh fp8e4m3 x128 | a_src f32 x4 | pad]
SB = 5  # blocks per gather superblock
SCRATCH = 32768
NQ = 4


def _pack_nodes(deg):
    """Snake-deal nodes (by degree desc) into NCORES*BLOCKS bins, <=128 each."""
    nbins = NCORES * BLOCKS
    order = np.argsort(-deg, kind="stable")
    nrounds = (N + nbins - 1) // nbins
    bin_of = np.empty(N, dtype=np.int64)
    slot_of = np.empty(N, dtype=np.int64)
    for r in range(nrounds):
        seg = order[r * nbins : (r + 1) * nbins]
        idx = np.arange(len(seg))
        b = idx if r % 2 == 0 else (nbins - 1 - idx)
        bin_of[seg] = b
        slot_of[seg] = r
    assert slot_of.max() < P
    return bin_of, slot_of


def _build_host_data(x, edge_index, batch, W, att_src, att_dst, bias, lin_w, lin_b):
    import ml_dtypes

    x = np.asarray(x, dtype=np.float32)
    ei = np.asarray(edge_index, dtype=np.int64)
    batch = np.asarray(batch, dtype=np.int64)
    W = np.ascontiguousarray(np.asarray(W, dtype=np.float32))
    att_src = np.asarray(att_src, dtype=np.float32)
    att_dst = np.asarray(att_dst, dtype=np.float32)
    bias = np.asarray(bias, dtype=np.float32)
    lin_w = np.asarray(lin_w, dtype=np.float32)
    lin_b = np.asarray(lin_b, dtype=np.float32)

    src_all = np.concatenate([ei[0], np.arange(N, dtype=np.int64)])
    dst_all = np.concatenate([ei[1], np.arange(N, dtype=np.int64)])
    E_tot = src_all.shape[0]

    deg_in = np.bincount(dst_all, minlength=N)
    bin_of, slot_of = _pack_nodes(deg_in)
    core_of = bin_of // BLOCKS
    block_of = bin_of % BLOCKS
    pos_of = block_of * P + slot_of  # [N] position within core shard

    e_core = core_of[dst_all]
    e_block = block_of[dst_all]
    e_chunk = core_of[src_all] // 2

    cnt = np.zeros((NCORES, BLOCKS, NCHUNK), dtype=np.int64)
    np.add.at(cnt, (e_core, e_block, e_chunk), 1)
    cap = T_BC_MIN * P
    # chunk-aware fixup: move nodes out of bins whose per-chunk load exceeds cap
    if cnt.max() > cap:
        nbins = NCORES * BLOCKS
        node_chunk_cnt = np.zeros((N, NCHUNK), dtype=np.int64)
        np.add.at(node_chunk_cnt, (dst_all, e_chunk), 1)
        bin_cnt = cnt.reshape(nbins, NCHUNK).copy()
        bin_fill = np.bincount(bin_of, minlength=nbins)
        nodes_by_bin = [list(np.where(bin_of == bb)[0]) for bb in range(nbins)]
        # same core-pair only: keeps every src's chunk assignment unchanged
        qgrp_of_bin = np.arange(nbins) // (2 * BLOCKS)
        for _ in range(200000):
            worst = int(np.argmax(bin_cnt.max(axis=1)))
            if bin_cnt[worst].max() <= cap:
                break
            cands = nodes_by_bin[worst]
            ci = int(np.argmax(bin_cnt[worst]))
            best_n = max(cands, key=lambda n: node_chunk_cnt[n, ci])
            headroom = cap - bin_cnt - node_chunk_cnt[best_n][None, :]
            ok = (
                (headroom.min(axis=1) >= 0)
                & (bin_fill < P)
                & (qgrp_of_bin == qgrp_of_bin[worst])
            )
            ok[worst] = False
            if not ok.any():
                break
            tgt = int(np.argmax(np.where(ok, headroom.min(axis=1), -(10**9))))
            nodes_by_bin[worst].remove(best_n)
            nodes_by_bin[tgt].append(best_n)
            bin_cnt[worst] -= node_chunk_cnt[best_n]
            bin_cnt[tgt] += node_chunk_cnt[best_n]
            bin_fill[worst] -= 1
            bin_fill[tgt] += 1
            bin_of[best_n] = tgt
        # recompute placement-dependent arrays
        for bb in range(nbins):
            ns = nodes_by_bin[bb]
            slot_of[ns] = np.arange(len(ns))
        core_of = bin_of // BLOCKS
        block_of = bin_of % BLOCKS
        pos_of = block_of * P + slot_of
        e_core = core_of[dst_all]
        e_block = block_of[dst_all]
        e_chunk = core_of[src_all] // 2
        cnt = np.zeros((NCORES, BLOCKS, NCHUNK), dtype=np.int64)
        np.add.at(cnt, (e_core, e_block, e_chunk), 1)
    t_bc = max(int(np.ceil(cnt.max() / P)), T_BC_MIN)
    T_BLK = t_bc * NCHUNK
    NTILES = BLOCKS * T_BLK
    SLOTS = NTILES * P
    RUN = t_bc * P

    run_base = (e_block * NCHUNK + e_chunk) * RUN
    key = (e_core * BLOCKS + e_block) * NCHUNK + e_chunk
    order = np.argsort(key, kind="stable")
    ks = key[order]
    run_start = np.searchsorted(ks, np.arange(NCORES * BLOCKS * NCHUNK))
    within = np.empty(E_tot, dtype=np.int64)
    within[order] = np.arange(E_tot) - run_start[ks]
    slot = run_base + within

    idx_src16 = np.full((NCORES, SLOTS), DUMMY_REL, dtype=np.int16)
    dstrel = np.full((NCORES, SLOTS), -7, dtype=np.int8)
    tblrow = core_of[src_all] * SHARD_ROWS + pos_of[src_all]
    idx_src16[e_core, slot] = (tblrow - e_chunk * CHUNK_ROWS).astype(np.int16)
    dstrel[e_core, slot] = (pos_of[dst_all] % P).astype(np.int8)

    # gather-window slot order: (sb, chunk) -> SB consecutive blocks' chunk runs
    NSB = BLOCKS // SB
    WIN_MAIN = SB * RUN
    win_flat = np.empty(NSB * NCHUNK * WIN_MAIN, dtype=np.int64)
    w = 0
    for s in range(NSB):
        for c in range(NCHUNK):
            for b in range(SB * s, SB * s + SB):
                base = (b * NCHUNK + c) * RUN
                win_flat[w : w + RUN] = np.arange(base, base + RUN)
                w += RUN
    assert w == SLOTS

    def wrap16(vals):
        n = vals.shape[0]
        wv = vals.reshape(n // 16, 16).T.astype(np.int16)
        return np.tile(wv, (8, 1))

    idx_src_w = np.zeros((NCORES, 128, SLOTS // 16), dtype=np.int16)
    dstrel_w = np.zeros((NCORES, 128, NTILES), dtype=np.int8)
    dstrel_row = np.zeros((NCORES, BLOCKS, 1, T_BLK * P), dtype=np.int8)
    for k in range(NCORES):
        idx_src_w[k] = wrap16(idx_src16[k][win_flat])
        dk = dstrel[k].reshape(NTILES, P)
        dstrel_w[k] = dk.T
        dstrel_row[k] = dk.reshape(BLOCKS, 1, T_BLK * P)

    # phase A inputs
    allpos = core_of * NPC + pos_of
    xk_flat = np.zeros(NCORES * NPC, dtype=np.int64)
    xk_flat[allpos] = np.arange(N)
    mask_flat = np.zeros(NCORES * NPC, dtype=bool)
    mask_flat[allpos] = True
    xk = xk_flat.reshape(NCORES, NPC)
    nodemask = mask_flat.reshape(NCORES, NPC)
    xT_tiles = np.zeros((NCORES, BLOCKS, FIN, P), dtype=ml_dtypes.bfloat16)
    for k in range(NCORES):
        xs = np.where(nodemask[k][:, None], x[xk[k]], 0.0)
        xT_tiles[k] = np.ascontiguousarray(
            xs.reshape(BLOCKS, P, FIN).transpose(0, 2, 1)
        ).astype(ml_dtypes.bfloat16)

    A8 = np.zeros((HC, 8), dtype=np.float32)
    for h in range(H):
        A8[h * C : (h + 1) * C, h] = att_src[h]
        A8[h * C : (h + 1) * C, 4 + h] = att_dst[h]

    graph_flat = np.zeros(NCORES * NPC, dtype=np.int64)
    graph_flat[allpos] = batch
    gf = graph_flat.reshape(NCORES, NPC)
    g_onehot = np.zeros((NCORES, BLOCKS, P, B), dtype=ml_dtypes.bfloat16)
    for k in range(NCORES):
        oh = (gf[k][:, None] == np.arange(B)[None, :]) & nodemask[k][:, None]
        g_onehot[k] = oh.reshape(BLOCKS, P, B).astype(ml_dtypes.bfloat16)

    counts = np.bincount(batch, minlength=B).astype(np.float32)
    counts_recip = (1.0 / np.maximum(counts, 1.0)).reshape(B, 1)

    dummy_row = np.zeros((4, REC_F32), dtype=np.float32)
    dummy_row[:, 32:36] = -1e30  # a_src of pad rows -> exp() == 0

    iota_bf = np.tile(np.arange(P), (P, 1)).reshape(P, 1, P)

    return dict(
        t_bc=t_bc,
        T_BLK=T_BLK,
        NTILES=NTILES,
        SLOTS=SLOTS,
        NSB=NSB,
        WIN_MAIN=WIN_MAIN,
        idx_src_w=idx_src_w,
        dstrel_w=dstrel_w,
        dstrel_row=dstrel_row,
        xT_tiles=xT_tiles,
        A8=A8,
        g_onehot=g_onehot,
        counts_recip=counts_recip,
        iota_bf=iota_bf.astype(np.int8),
        iota_col=np.arange(P, dtype=np.int8).reshape(P, 1),
        bias_np=bias,
        bias_rep=np.tile(bias[None, :], (P, 1)).astype(np.float32),
        linb_rep=np.tile(lin_b[None, :], (B, 1)).astype(np.float32),
        identity=np.eye(P, dtype=np.float32),
        dummy_row=dummy_row,
        W=W,
        linwT=np.ascontiguousarray(lin_w.T),
    )


def _build_nc(hp):
    import concourse.bacc as bacc
    import concourse.bass as bass
    import concourse.mybir as mybir
    import concourse.tile as tile

    t_bc = hp["t_bc"]
    T_BLK = hp["T_BLK"]
    NTILES = hp["NTILES"]
    SLOTS = hp["SLOTS"]
    NSB = hp["NSB"]
    WIN_MAIN = hp["WIN_MAIN"]
    WIN_T = SB * t_bc
    with_bias = bool(np.any(hp["bias_np"]))
    dt = mybir.dt
    alu = mybir.AluOpType
    act = mybir.ActivationFunctionType

    nc = bacc.Bacc(
        None,
        target_bir_lowering=False,
        debug=False,
        num_swdge_queues=NQ,
        dynamic_dma_scratch_size=SCRATCH,
        num_devices=NCORES,
    )

    xT_in = nc.dram_tensor("xT_in", [BLOCKS, FIN, P], dt.bfloat16, kind="ExternalInput")
    W_in = nc.dram_tensor("W_in", [FIN, HC], dt.float32, kind="ExternalInput")
    A8_in = nc.dram_tensor("A8_in", [HC, 8], dt.float32, kind="ExternalInput")
    ident_in = nc.dram_tensor("ident_in", [P, P], dt.float32, kind="ExternalInput")
    iota_in = nc.dram_tensor("iota_in", [P, 1, P], dt.int8, kind="ExternalInput")
    iotac_in = nc.dram_tensor("iotac_in", [P, 1], dt.int8, kind="ExternalInput")
    bias_in = nc.dram_tensor("bias_in", [P, HC], dt.float32, kind="ExternalInput")
    dummy_in = nc.dram_tensor(
        "dummy_in", [4, REC_F32], dt.float32, kind="ExternalInput"
    )
    isrc_in = nc.dram_tensor(
        "isrc_in", [128, SLOTS // 16], dt.int16, kind="ExternalInput"
    )
    dstrel_in = nc.dram_tensor(
        "dstrel_in", [128, NTILES], dt.int8, kind="ExternalInput"
    )
    drow_in = nc.dram_tensor(
        "drow_in", [BLOCKS, 1, T_BLK * P], dt.int8, kind="ExternalInput"
    )
    goh_in = nc.dram_tensor("goh_in", [BLOCKS, P, B], dt.bfloat16, kind="ExternalInput")
    crecip_in = nc.dram_tensor("crecip_in", [B, 1], dt.float32, kind="ExternalInput")
    linw_in = nc.dram_tensor("linw_in", [HC, NCLS], dt.float32, kind="ExternalInput")
    linb_in = nc.dram_tensor("linb_in", [B, NCLS], dt.float32, kind="ExternalInput")
    out_fin = nc.dram_tensor("out_fin", [B, NCLS], dt.float32, kind="ExternalOutput")

    shard = nc.dram_tensor(
        "shard", [SHARD_ROWS, REC_F32], dt.float32, kind="Internal"
    )
    table = nc.dram_tensor(
        "table", [TBL_ROWS, REC_F32], dt.float32, kind="Internal", addr_space="Shared"
    )
    pool_in = nc.dram_tensor("pool_in", [B, HC], dt.float32, kind="Internal")
    pool_out = nc.dram_tensor(
        "pool_out", [B, HC], dt.float32, kind="Internal", addr_space="Shared"
    )

    with tile.TileContext(nc) as tc:
        with (
            tc.tile_pool(name="const", bufs=1) as constp,
            tc.tile_pool(name="na", bufs=3) as nap,
            tc.tile_pool(name="ps", bufs=1, space="PSUM") as psp,
            tc.tile_pool(name="gw", bufs=1) as gwp,
            tc.tile_pool(name="ix", bufs=1) as ixp,
            tc.tile_pool(name="oh", bufs=1) as ohp,
            tc.tile_pool(name="ed", bufs=2) as edp,
            tc.tile_pool(name="fl", bufs=2) as flp,
        ):
            iota = constp.tile([P, 1, P], dt.int8)
            nc.sync.dma_start(iota[:], iota_in[:])
            iotac = constp.tile([P, 1], dt.int8)
            nc.sync.dma_start(iotac[:], iotac_in[:])
            ident = constp.tile([P, P], dt.float32)
            nc.sync.dma_start(ident[:], ident_in[:])
            w_t = constp.tile([FIN, HC], dt.float32)
            nc.sync.dma_start(w_t[:], W_in[:])
            a8_t = constp.tile([HC, 8], dt.float32)
            nc.sync.dma_start(a8_t[:], A8_in[:])
            dstrel_t = constp.tile([128, NTILES], dt.int8)
            nc.sync.dma_start(dstrel_t[:], dstrel_in[:])
            if with_bias:
                bias_t = constp.tile([P, HC], dt.float32)
                nc.sync.dma_start(bias_t[:], bias_in[:])

            # rhs_all = [W | W@A8] in bf16
            wT_ps = psp.tile([HC, FIN], dt.float32, space="PSUM", tag="misc", bufs=1)
            nc.tensor.transpose(wT_ps[:], w_t[:], ident[:])
            wT_sb = nap.tile([HC, FIN], dt.float32, tag="wt")
            nc.vector.tensor_copy(wT_sb[:], wT_ps[:])
            wsc_ps = psp.tile([FIN, 8], dt.float32, space="PSUM", tag="misc", bufs=1)
            nc.tensor.matmul(
                wsc_ps[:], lhsT=wT_sb[:], rhs=a8_t[:], start=True, stop=True
            )
            rhs_all = nap.tile([FIN, HC + 8], dt.bfloat16, tag="rhsall", bufs=1)
            nc.vector.tensor_copy(rhs_all[:, 0:HC], w_t[:])
            nc.vector.tensor_copy(rhs_all[:, HC : HC + 8], wsc_ps[:])

            nc.sync.dma_start(shard[NPC:SHARD_ROWS, :], dummy_in[0:4])

            # persistent a_dst table: [128, BLOCKS*4] bf16
            adst_sb = constp.tile([P, BLOCKS * 4], dt.bfloat16)

            # ---------- phase A + pipelined AllGather quarters ----------
            for t in range(BLOCKS):
                xt = nap.tile([FIN, P], dt.bfloat16, tag="xt")
                nc.sync.dma_start(xt[:], xT_in[t])
                aps = psp.tile(
                    [P, HC + 8], dt.float32, space="PSUM", tag="aps", bufs=2
                )
                nc.tensor.matmul(
                    aps[:], lhsT=xt[:], rhs=rhs_all[:], start=True, stop=True
                )
                rec = nap.tile([P, REC_F32], dt.float32, tag="rec")
                rec8 = rec[:].bitcast(dt.float8e4)
                nc.vector.tensor_copy(rec8[:, 0:HC], aps[:, 0:HC])
                nc.scalar.activation(
                    rec[:, 32:36], aps[:, HC : HC + 4], act.Copy
                )
                nc.scalar.activation(
                    adst_sb[:, t * 4 : (t + 1) * 4], aps[:, HC + 4 : HC + 8], act.Copy
                )
                nc.sync.dma_start(shard[t * P : (t + 1) * P, :], rec[:, :])

            # ---------- phase B ----------
            nc.gpsimd.collective_compute(
                "AllGather",
                alu.bypass,
                replica_groups=[list(range(NCORES))],
                ins=[shard[:, :]],
                outs=[table[:, :]],
            )

            # ---------- phase C ----------
            pooled_ps = psp.tile([B, HC], dt.float32, space="PSUM", tag="pool", bufs=1)

            # software pipeline state
            pend = {}  # b -> dict(acc, goh, srec, srec2, outb, outb2, outbf)
            prep = {}  # b -> (oh, adst_ps)

            def issue_drep(b):
                drep = ohp.tile([P, T_BLK, P], dt.int8, tag="drep", bufs=3)
                nc.sync.dma_start(
                    drep[:].rearrange("p t q -> p (t q)"),
                    drow_in[b].to_broadcast((P, T_BLK * P)),
                )
                return drep

            def build_prep(b, drep):
                ohT = ohp.tile([P, T_BLK, P], dt.bfloat16, tag="ohT", bufs=2)
                nc.vector.tensor_tensor(
                    out=ohT[:],
                    in0=drep[:],
                    in1=iotac[:]
                    .rearrange("p x -> p x ()")
                    .to_broadcast((P, T_BLK, P)),
                    op=alu.is_equal,
                )
                oh = ohp.tile([P, T_BLK, P], dt.bfloat16, tag="oh", bufs=2)
                nc.vector.tensor_tensor(
                    out=oh[:],
                    in0=dstrel_t[:, b * T_BLK : (b + 1) * T_BLK]
                    .rearrange("p t -> p t ()")
                    .to_broadcast((P, T_BLK, P)),
                    in1=iota[:].to_broadcast((P, T_BLK, P)),
                    op=alu.is_equal,
                )
                adst_ps = psp.tile(
                    [P, T_BLK, 4], dt.float32, space="PSUM", tag="adst", bufs=2
                )
                for t in range(T_BLK):
                    nc.tensor.matmul(
                        adst_ps[:, t, :],
                        lhsT=ohT[:, t, :],
                        rhs=adst_sb[:, b * 4 : (b + 1) * 4],
                        start=True,
                        stop=True,
                    )
                prep[b] = (oh, adst_ps)

            def flush_dve1(b):
                d = pend[b]
                srec = flp.tile([P, 4], dt.float32, tag="srec")
                nc.vector.tensor_scalar(
                    out=srec[:],
                    in0=d["acc"][:, HC : HC + 4],
                    scalar1=1e-30,
                    scalar2=None,
                    op0=alu.max,
                )
                nc.vector.reciprocal(srec[:], srec[:])
                d["srec"] = srec

            def flush_scalar(b):
                d = pend[b]
                if with_bias:
                    return
                outb = flp.tile([P, HC], dt.float32, tag="outb")
                outb2 = flp.tile([P, HC], dt.float32, tag="outb2")
                for h in range(H):
                    nc.scalar.activation(
                        outb[:, h * C : (h + 1) * C],
                        d["acc"][:, h * C : (h + 1) * C],
                        act.Copy,
                        scale=d["srec"][:, h : h + 1],
                    )
                nc.scalar.activation(outb2[:], outb[:], act.Copy, scale=NEG_ACT)
                d["outb"], d["outb2"] = outb, outb2

            def flush_dve2(b):
                d = pend[b]
                outbf = flp.tile([P, HC], dt.bfloat16, tag="outbf")
                if with_bias:
                    outb = flp.tile([P, HC], dt.float32, tag="outb")
                    nc.vector.tensor_tensor(
                        out=outb[:].rearrange("p (h c) -> p h c", h=H),
                        in0=d["acc"][:, 0:HC].rearrange("p (h c) -> p h c", h=H),
                        in1=d["srec"][:]
                        .rearrange("p h -> p h ()")
                        .to_broadcast((P, H, C)),
                        op=alu.mult,
                    )
                    nc.vector.tensor_add(outb[:], outb[:], bias_t[:])
                    tmpo = flp.tile([P, HC], dt.float32, tag="tmpo")
                    nc.vector.tensor_scalar_mul(tmpo[:], outb[:], NEG_ACT)
                    nc.vector.tensor_tensor(
                        out=outbf[:], in0=outb[:], in1=tmpo[:], op=alu.max
                    )
                else:
                    nc.vector.tensor_tensor(
                        out=outbf[:], in0=d["outb"][:], in1=d["outb2"][:], op=alu.max
                    )
                d["outbf"] = outbf

            def flush_pool(b):
                d = pend.pop(b)
                nc.tensor.matmul(
                    pooled_ps[:],
                    lhsT=d["goh"][:],
                    rhs=d["outbf"][:],
                    start=(b == 0),
                    stop=(b == BLOCKS - 1),
                )

            # prologue: drep(0), drep(1), prep(0)
            dreps = {0: issue_drep(0), 1: issue_drep(1)}
            build_prep(0, dreps.pop(0))
            for s in range(NSB):
                gwin = gwp.tile(
                    [P, NCHUNK, WIN_T, REC_F32], dt.float32, tag="gwin", bufs=3
                )
                for c in range(NCHUNK):
                    off = (s * NCHUNK + c) * WIN_MAIN
                    ix1 = ixp.tile([128, WIN_MAIN // 16], dt.int16, tag="ix1", bufs=8)
                    nc.sync.dma_start(
                        ix1[:], isrc_in[:, off // 16 : (off + WIN_MAIN) // 16]
                    )
                    nc.gpsimd.dma_gather(
                        out_ap=gwin[:, c, :, :],
                        in_ap=table[c * CHUNK_ROWS : (c + 1) * CHUNK_ROWS, :],
                        idxs_ap=ix1[:],
                        num_idxs=WIN_MAIN,
                        num_idxs_reg=WIN_MAIN,
                        elem_size=REC_F32,
                        single_packet=False,
                        queue_num=c % NQ,
                    )

                for bb in range(SB):
                    b = s * SB + bb
                    sl = slice(bb * t_bc, (bb + 1) * t_bc)
                    oh, adst_ps = prep.pop(b)
                    pblk = edp.tile([P, T_BLK, 4], dt.float32, tag="pblk")
                    nc.vector.tensor_tensor(
                        out=pblk[:].rearrange("p (c t) h -> p c t h", c=NCHUNK),
                        in0=gwin[:, :, sl, 32:36],
                        in1=adst_ps[:].rearrange("p (c t) h -> p c t h", c=NCHUNK),
                        op=alu.add,
                    )
                    # exp(leaky_s(x)) == max(exp(x), exp(s*x)) for 0<s<1
                    pexp1 = edp.tile([P, T_BLK, 4], dt.float32, tag="pexp1")
                    nc.scalar.activation(pexp1[:], pblk[:], act.Exp)
                    pexp2 = edp.tile([P, T_BLK, 4], dt.float32, tag="pexp2")
                    nc.scalar.activation(pexp2[:], pblk[:], act.Exp, scale=NEG_ATT)
                    # DVE fillers while the scalar exps run
                    if b > 0:
                        flush_dve1(b - 1)
                    if b + 2 < BLOCKS:
                        dreps[b + 2] = issue_drep(b + 2)
                    if b + 1 < BLOCKS:
                        build_prep(b + 1, dreps.pop(b + 1))
                    pbf = edp.tile([P, T_BLK, 4], dt.bfloat16, tag="pbf")
                    nc.vector.tensor_tensor(
                        out=pbf[:], in0=pexp1[:], in1=pexp2[:], op=alu.max
                    )

                    acc = psp.tile(
                        [P, HC + 4], dt.float32, space="PSUM", tag="acc", bufs=2
                    )
                    msg = edp.tile(
                        [P, T_BLK, HC + 4], dt.bfloat16, tag="msg", bufs=3
                    )
                    nc.vector.tensor_copy(msg[:, :, HC : HC + 4], pbf[:])
                    gb = gwin[:, :, sl, :].bitcast(dt.float8e4)
                    for c in range(NCHUNK):
                        slp = slice(c * t_bc, (c + 1) * t_bc)
                        nc.vector.tensor_tensor(
                            out=msg[:, slp, 0:HC].rearrange(
                                "p t (h c2) -> p t h c2", h=H
                            ),
                            in0=gb[:, c, :, 0:HC].rearrange(
                                "p t (h c2) -> p t h c2", h=H
                            ),
                            in1=pbf[:, slp, :]
                            .rearrange("p t h -> p t h ()")
                            .to_broadcast((P, t_bc, H, C)),
                            op=alu.mult,
                        )
                    for t_in_blk in range(T_BLK):
                        nc.tensor.matmul(
                            acc[:],
                            lhsT=oh[:, t_in_blk, :],
                            rhs=msg[:, t_in_blk, :],
                            start=(t_in_blk == 0),
                            stop=(t_in_blk == T_BLK - 1),
                        )
                    goh = flp.tile([P, B], dt.bfloat16, tag="goh")
                    nc.sync.dma_start(goh[:], goh_in[b])
                    pend[b] = {"acc": acc, "goh": goh}
                    if b > 0:
                        flush_scalar(b - 1)
                        flush_dve2(b - 1)
                        flush_pool(b - 1)
            flush_dve1(BLOCKS - 1)
            flush_scalar(BLOCKS - 1)
            flush_dve2(BLOCKS - 1)
            flush_pool(BLOCKS - 1)

            # ---------- pooling + final linear ----------
            pooled_sb = nap.tile([B, HC], dt.float32, tag="poolsb", bufs=1)
            nc.vector.tensor_copy(pooled_sb[:], pooled_ps[:])
            nc.sync.dma_start(pool_in[:], pooled_sb[:])
            nc.gpsimd.collective_compute(
                "AllReduce",
                alu.add,
                replica_groups=[list(range(NCORES))],
                ins=[pool_in[:]],
                outs=[pool_out[:]],
            )
            pooled2 = nap.tile([B, HC], dt.float32, tag="pool2", bufs=1)
            nc.sync.dma_start(pooled2[:], pool_out[:])
            crecip = nap.tile([B, 1], dt.float32, tag="crecip", bufs=1)
            nc.sync.dma_start(crecip[:], crecip_in[:])
            nc.vector.tensor_scalar(
                out=pooled2[:],
                in0=pooled2[:],
                scalar1=crecip[:],
                scalar2=None,
                op0=alu.mult,
            )
            p2T_ps = psp.tile([HC, B], dt.float32, space="PSUM", tag="misc", bufs=1)
            nc.tensor.transpose(p2T_ps[:], pooled2[:], ident[:])
            p2T = nap.tile([HC, B], dt.float32, tag="p2T", bufs=1)
            nc.vector.tensor_copy(p2T[:], p2T_ps[:])
            linw_t = nap.tile([HC, NCLS], dt.float32, tag="linw", bufs=1)
            nc.sync.dma_start(linw_t[:], linw_in[:])
            fin_ps = psp.tile([B, NCLS], dt.float32, space="PSUM", tag="misc", bufs=1)
            nc.tensor.matmul(
                fin_ps[:], lhsT=p2T[:], rhs=linw_t[:], start=True, stop=True
            )
            fin_sb = nap.tile([B, NCLS], dt.float32, tag="finsb", bufs=1)
            nc.vector.tensor_copy(fin_sb[:], fin_ps[:])
            linb_t = nap.tile([B, NCLS], dt.float32, tag="linb", bufs=1)
            nc.sync.dma_start(linb_t[:], linb_in[:])
            nc.vector.tensor_add(fin_sb[:], fin_sb[:], linb_t[:])
            nc.sync.dma_start(out_fin[:], fin_sb[:])

    nc.compile()
    return nc


def _in_maps(hp):
    maps = []
    for k in range(NCORES):
        maps.append(
            {
                "xT_in": hp["xT_tiles"][k],
                "W_in": hp["W"],
                "A8_in": hp["A8"],
                "ident_in": hp["identity"],
                "iota_in": hp["iota_bf"],
                "iotac_in": hp["iota_col"],
                "bias_in": hp["bias_rep"],
                "dummy_in": hp["dummy_row"],
                "isrc_in": hp["idx_src_w"][k],
                "dstrel_in": hp["dstrel_w"][k],
                "drow_in": hp["dstrel_row"][k],
                "goh_in": hp["g_onehot"][k],
                "crecip_in": hp["counts_recip"],
                "linw_in": hp["linwT"],
                "linb_in": hp["linb_rep"],
            }
        )
    return maps


def kernel(x, edge_index, batch, batch_size, W, att_src, att_dst, bias, lin_w, lin_b):
    hp = _build_host_data(x, edge_index, batch, W, att_src, att_dst, bias, lin_w, lin_b)
    nc = _build_nc(hp)
    from concourse.bass_utils import run_bass_kernel_spmd

    res = run_bass_kernel_spmd(
        nc, _in_maps(hp), core_ids=list(range(NCORES)), trace=False
    )
    return np.asarray(res.results[0]["out_fin"], dtype=np.float32)


# revision 47
# speedup vs baseline: 1.0122x; 1.0122x over previous
"""Distributed GAT (GATConv eval + global mean pool + linear) on 8 TRN2 NeuronCores.

Pipeline (shapes hardcoded for nn_GAT_27968827032308):
  Host: renumber nodes -> 8 cores x 100 blocks x 128 slots (degree-balanced deal
    packing); route each edge (incl. self-loops) to the core owning its dst;
    order per-core edges [block, src-chunk], pad runs to 128-slot tiles; build
    int16 gather-index stream plus per-tile dst-onehot scalar streams.
  Phase A (device, per core): records for own nodes: xh = x@W (PE, bf16),
    a_src/a_dst = x@(W@[Asrc|Adst]); 512B record rows
    [xh bf16 x128 | a_src f32 x4 | pad] -> DRAM shard; a_dst kept in a
    persistent SBUF table [128, BLOCKS*4] bf16 (dst side is always local).
  Phase B: AllGather shards -> replicated full record table.
  Phase C (per 5-block superblock): one dma_gather per src-chunk pulls
    25 record tiles; per block: onehot oh[slot,node] and transposed onehot
    ohT[node,slot] built in single batched DVE ops; a_dst delivered to edge
    slots via per-tile PE matmuls ohT^T @ adst_blk (no second gather);
    p = exp(leaky_0.2(a_src+a_dst)) on the Scalar engine; batched 4D msg
    multiply; PE matmul acc[n,0:128] += oh^T @ (p*xh), acc[n,128:132] += oh^T@p
    accumulated in PSUM per node-block; flush via Scalar Lrelu(acc*1/s).
  Pooling: PE matmul with host-built graph-onehot -> partial pooled sums
    [128 graphs, 128]; AllReduce; * (1/count); final linear via PE.

kernel(**inputs): FULL inputs -> FULL [128, 10] float32 output.
"""

import sys

sys.path.insert(0, "/opt/trn_rl_repo")
sys.path.insert(0, "/opt/trn_rl_repo/concourse")

import numpy as np

N = 100000
FIN = 128
H = 4
C = 32
HC = H * C
B = 128
NCLS = 10
NEG_ATT = 0.2
NEG_ACT = 0.01

NCORES = 8
BLOCKS = 100
P = 128
NPC = BLOCKS * P  # 12800 padded node slots per core
T_BC_MIN = 5
NCHUNK = 4
SHARD_ROWS = NPC + 4  # + pad rows; dummy record at SHARD_ROWS-1
CHUNK_ROWS = 2 * SHARD_ROWS  # 25608 < 32768 (int16 gather idx)
TBL_ROWS = NCORES * SHARD_ROWS
DUMMY_REL = SHARD_ROWS - 1  # same rel for every chunk (core 2c's dummy row)
REC_F32 = 64  # 256B records: [xh fp8e4m3 x128 | a_src f32 x4 | pad]
SB = 5  # blocks per gather superblock
SCRATCH = 32768
NQ = 4


def _pack_nodes(deg):
    """Snake-deal nodes (by degree desc) into NCORES*BLOCKS bins, <=128 each."""
    nbins = NCORES * BLOCKS
    order = np.argsort(-deg, kind="stable")
    nrounds = (N + nbins - 1) // nbins
    bin_of = np.empty(N, dtype=np.int64)
    slot_of = np.empty(N, dtype=np.int64)
    for r in range(nrounds):
        seg = order[r * nbins : (r + 1) * nbins]
        idx = np.arange(len(seg))
        b = idx if r % 2 == 0 else (nbins - 1 - idx)
        bin_of[seg] = b
        slot_of[seg] = r
    assert slot_of.max() < P
    return bin_of, slot_of


def _build_host_data(x, edge_index, batch, W, att_src, att_dst, bias, lin_w, lin_b):
    import ml_dtypes

    x = np.asarray(x, dtype=np.float32)
    ei = np.asarray(edge_index, dtype=np.int64)
    batch = np.asarray(batch, dtype=np.int64)
    W = np.ascontiguousarray(np.asarray(W, dtype=np.float32))
    att_src = np.asarray(att_src, dtype=np.float32)
    att_dst = np.asarray(att_dst, dtype=np.float32)
    bias = np.asarray(bias, dtype=np.float32)
    lin_w = np.asarray(lin_w, dtype=np.float32)
    lin_b = np.asarray(lin_b, dtype=np.float32)

    src_all = np.concatenate([ei[0], np.arange(N, dtype=np.int64)])
    dst_all = np.concatenate([ei[1], np.arange(N, dtype=np.int64)])
    E_tot = src_all.shape[0]

    deg_in = np.bincount(dst_all, minlength=N)
    bin_of, slot_of = _pack_nodes(deg_in)
    core_of = bin_of // BLOCKS
    block_of = bin_of % BLOCKS
    pos_of = block_of * P + slot_of  # [N] position within core shard

    e_core = core_of[dst_all]
    e_block = block_of[dst_all]
    e_chunk = core_of[src_all] // 2

    cnt = np.zeros((NCORES, BLOCKS, NCHUNK), dtype=np.int64)
    np.add.at(cnt, (e_core, e_block, e_chunk), 1)
    cap = T_BC_MIN * P
    # chunk-aware fixup: move nodes out of bins whose per-chunk load exceeds cap
    if cnt.max() > cap:
        nbins = NCORES * BLOCKS
        node_chunk_cnt = np.zeros((N, NCHUNK), dtype=np.int64)
        np.add.at(node_chunk_cnt, (dst_all, e_chunk), 1)
        bin_cnt = cnt.reshape(nbins, NCHUNK).copy()
        bin_fill = np.bincount(bin_of, minlength=nbins)
        nodes_by_bin = [list(np.where(bin_of == bb)[0]) for bb in range(nbins)]
        # same core-pair only: keeps every src's chunk assignment unchanged
        qgrp_of_bin = np.arange(nbins) // (2 * BLOCKS)
        for _ in range(200000):
            worst = int(np.argmax(bin_cnt.max(axis=1)))
            if bin_cnt[worst].max() <= cap:
                break
            cands = nodes_by_bin[worst]
            ci = int(np.argmax(bin_cnt[worst]))
            best_n = max(cands, key=lambda n: node_chunk_cnt[n, ci])
            headroom = cap - bin_cnt - node_chunk_cnt[best_n][None, :]
            ok = (
                (headroom.min(axis=1) >= 0)
                & (bin_fill < P)
                & (qgrp_of_bin == qgrp_of_bin[worst])
            )
            ok[worst] = False
            if not ok.any():
                break
            tgt = int(np.argmax(np.where(ok, headroom.min(axis=1), -(10**9))))
            nodes_by_bin[worst].remove(best_n)
            nodes_by_bin[tgt].append(best_n)
            bin_cnt[worst] -= node_chunk_cnt[best_n]
            bin_cnt[tgt] += node_chunk_cnt[best_n]
            bin_fill[worst] -= 1
            bin_fill[tgt] += 1
            bin_of[best_n] = tgt
        # recompute placement-dependent arrays
        for bb in range(nbins):
            ns = nodes_by_bin[bb]
            slot_of[ns] = np.arange(len(ns))
        core_of = bin_of // BLOCKS
        block_of = bin_of % BLOCKS
        pos_of = block_of * P + slot_of
        e_core = core_of[dst_all]
        e_block = block_of[dst_all]
        e_chunk = core_of[src_all] // 2
        cnt = np.zeros((NCORES, BLOCKS, NCHUNK), dtype=np.int64)
        np.add.at(cnt, (e_core, e_block, e_chunk), 1)
    t_bc = max(int(np.ceil(cnt.max() / P)), T_BC_MIN)
    T_BLK = t_bc * NCHUNK
    NTILES = BLOCKS * T_BLK
    SLOTS = NTILES * P
    RUN = t_bc * P

    run_base = (e_block * NCHUNK + e_chunk) * RUN
    key = (e_core * BLOCKS + e_block) * NCHUNK + e_chunk
    order = np.argsort(key, kind="stable")
    ks = key[order]
    run_start = np.searchsorted(ks, np.arange(NCORES * BLOCKS * NCHUNK))
    within = np.empty(E_tot, dtype=np.int64)
    within[order] = np.arange(E_tot) - run_start[ks]
    slot = run_base + within

    idx_src16 = np.full((NCORES, SLOTS), DUMMY_REL, dtype=np.int16)
    dstrel = np.full((NCORES, SLOTS), -7, dtype=np.int8)
    tblrow = core_of[src_all] * SHARD_ROWS + pos_of[src_all]
    idx_src16[e_core, slot] = (tblrow - e_chunk * CHUNK_ROWS).astype(np.int16)
    dstrel[e_core, slot] = (pos_of[dst_all] % P).astype(np.int8)

    # gather-window slot order: (sb, chunk) -> SB consecutive blocks' chunk runs
    NSB = BLOCKS // SB
    WIN_MAIN = SB * RUN
    win_flat = np.empty(NSB * NCHUNK * WIN_MAIN, dtype=np.int64)
    w = 0
    for s in range(NSB):
        for c in range(NCHUNK):
            for b in range(SB * s, SB * s + SB):
                base = (b * NCHUNK + c) * RUN
                win_flat[w : w + RUN] = np.arange(base, base + RUN)
                w += RUN
    assert w == SLOTS

    def wrap16(vals):
        n = vals.shape[0]
        wv = vals.reshape(n // 16, 16).T.astype(np.int16)
        return np.tile(wv, (8, 1))

    idx_src_w = np.zeros((NCORES, 128, SLOTS // 16), dtype=np.int16)
    dstrel_w = np.zeros((NCORES, 128, NTILES), dtype=np.int8)
    dstrel_row = np.zeros((NCORES, BLOCKS, 1, T_BLK * P), dtype=np.int8)
    for k in range(NCORES):
        idx_src_w[k] = wrap16(idx_src16[k][win_flat])
        dk = dstrel[k].reshape(NTILES, P)
        dstrel_w[k] = dk.T
        dstrel_row[k] = dk.reshape(BLOCKS, 1, T_BLK * P)

    # phase A inputs
    allpos = core_of * NPC + pos_of
    xk_flat = np.zeros(NCORES * NPC, dtype=np.int64)
    xk_flat[allpos] = np.arange(N)
    mask_flat = np.zeros(NCORES * NPC, dtype=bool)
    mask_flat[allpos] = True
    xk = xk_flat.reshape(NCORES, NPC)
    nodemask = mask_flat.reshape(NCORES, NPC)
    xT_tiles = np.zeros((NCORES, BLOCKS, FIN, P), dtype=ml_dtypes.bfloat16)
    for k in range(NCORES):
        xs = np.where(nodemask[k][:, None], x[xk[k]], 0.0)
        xT_tiles[k] = np.ascontiguousarray(
            xs.reshape(BLOCKS, P, FIN).transpose(0, 2, 1)
        ).astype(ml_dtypes.bfloat16)

    A8 = np.zeros((HC, 8), dtype=np.float32)
    for h in range(H):
        A8[h * C : (h + 1) * C, h] = att_src[h]
        A8[h * C : (h + 1) * C, 4 + h] = att_dst[h]

    graph_flat = np.zeros(NCORES * NPC, dtype=np.int64)
    graph_flat[allpos] = batch
    gf = graph_flat.reshape(NCORES, NPC)
    g_onehot = np.zeros((NCORES, BLOCKS, P, B), dtype=ml_dtypes.bfloat16)
    for k in range(NCORES):
        oh = (gf[k][:, None] == np.arange(B)[None, :]) & nodemask[k][:, None]
        g_onehot[k] = oh.reshape(BLOCKS, P, B).astype(ml_dtypes.bfloat16)

    counts = np.bincount(batch, minlength=B).astype(np.float32)
    counts_recip = (1.0 / np.maximum(counts, 1.0)).reshape(B, 1)

    dummy_row = np.zeros((4, REC_F32), dtype=np.float32)
    dummy_row[:, 32:36] = -1e30  # a_src of pad rows -> exp() == 0

    iota_bf = np.tile(np.arange(P), (P, 1)).reshape(P, 1, P)

    return dict(
        t_bc=t_bc,
        T_BLK=T_BLK,
        NTILES=NTILES,
        SLOTS=SLOTS,
        NSB=NSB,
        WIN_MAIN=WIN_MAIN,
        idx_src_w=idx_src_w,
        dstrel_w=dstrel_w,
        dstrel_row=dstrel_row,
        xT_tiles=xT_tiles,
        A8=A8,
        g_onehot=g_onehot,
        counts_recip=counts_recip,
        iota_bf=iota_bf.astype(np.int8),
        iota_col=np.arange(P, dtype=np.int8).reshape(P, 1),
        bias_np=bias,
        bias_rep=np.tile(bias[None, :], (P, 1)).astype(np.float32),
        linb_rep=np.tile(lin_b[None, :], (B, 1)).astype(np.float32),
        identity=np.eye(P, dtype=np.float32),
        dummy_row=dummy_row,
        W=W,
        linwT=np.ascontiguousarray(lin_w.T),
    )


def _build_nc(hp):
    import concourse.bacc as bacc
    import concourse.bass as bass
    import concourse.mybir as mybir
    import concourse.tile as tile

    t_bc = hp["t_bc"]
    T_BLK = hp["T_BLK"]
    NTILES = hp["NTILES"]
    SLOTS = hp["SLOTS"]
    NSB = hp["NSB"]
    WIN_MAIN = hp["WIN_MAIN"]
    WIN_T = SB * t_bc
    with_bias = bool(np.any(hp["bias_np"]))
    dt = mybir.dt
    alu = mybir.AluOpType
    act = mybir.ActivationFunctionType

    nc = bacc.Bacc(
        None,
        target_bir_lowering=False,
        debug=False,
        num_swdge_queues=NQ,
        dynamic_dma_scratch_size=SCRATCH,
        num_devices=NCORES,
    )

    xT_in = nc.dram_tensor("xT_in", [BLOCKS, FIN, P], dt.bfloat16, kind="ExternalInput")
    W_in = nc.dram_tensor("W_in", [FIN, HC], dt.float32, kind="ExternalInput")
    A8_in = nc.dram_tensor("A8_in", [HC, 8], dt.float32, kind="ExternalInput")
    ident_in = nc.dram_tensor("ident_in", [P, P], dt.float32, kind="ExternalInput")
    iota_in = nc.dram_tensor("iota_in", [P, 1, P], dt.int8, kind="ExternalInput")
    iotac_in = nc.dram_tensor("iotac_in", [P, 1], dt.int8, kind="ExternalInput")
    bias_in = nc.dram_tensor("bias_in", [P, HC], dt.float32, kind="ExternalInput")
    dummy_in = nc.dram_tensor(
        "dummy_in", [4, REC_F32], dt.float32, kind="ExternalInput"
    )
    isrc_in = nc.dram_tensor(
        "isrc_in", [128, SLOTS // 16], dt.int16, kind="ExternalInput"
    )
    dstrel_in = nc.dram_tensor(
        "dstrel_in", [128, NTILES], dt.int8, kind="ExternalInput"
    )
    drow_in = nc.dram_tensor(
        "drow_in", [BLOCKS, 1, T_BLK * P], dt.int8, kind="ExternalInput"
    )
    goh_in = nc.dram_tensor("goh_in", [BLOCKS, P, B], dt.bfloat16, kind="ExternalInput")
    crecip_in = nc.dram_tensor("crecip_in", [B, 1], dt.float32, kind="ExternalInput")
    linw_in = nc.dram_tensor("linw_in", [HC, NCLS], dt.float32, kind="ExternalInput")
    linb_in = nc.dram_tensor("linb_in", [B, NCLS], dt.float32, kind="ExternalInput")
    out_fin = nc.dram_tensor("out_fin", [B, NCLS], dt.float32, kind="ExternalOutput")

    shard = nc.dram_tensor(
        "shard", [SHARD_ROWS, REC_F32], dt.float32, kind="Internal"
    )
    table = nc.dram_tensor(
        "table", [TBL_ROWS, REC_F32], dt.float32, kind="Internal", addr_space="Shared"
    )
    pool_in = nc.dram_tensor("pool_in", [B, HC], dt.float32, kind="Internal")
    pool_out = nc.dram_tensor(
        "pool_out", [B, HC], dt.float32, kind="Internal", addr_space="Shared"
    )

    with tile.TileContext(nc) as tc:
        with (
            tc.tile_pool(name="const", bufs=1) as constp,
            tc.tile_pool(name="na", bufs=3) as nap,
            tc.tile_pool(name="ps", bufs=1, space="PSUM") as psp,
            tc.tile_pool(name="gw", bufs=1) as gwp,
            tc.tile_pool(name="ix", bufs=1) as ixp,
            tc.tile_pool(name="oh", bufs=1) as ohp,
            tc.tile_pool(name="ed", bufs=2) as edp,
            tc.tile_pool(name="fl", bufs=2) as flp,
        ):
            iota = constp.tile([P, 1, P], dt.int8)
            nc.sync.dma_start(iota[:], iota_in[:])
            iotac = constp.tile([P, 1], dt.int8)
            nc.sync.dma_start(iotac[:], iotac_in[:])
            ident = constp.tile([P, P], dt.float32)
            nc.sync.dma_start(ident[:], ident_in[:])
            w_t = constp.tile([FIN, HC], dt.float32)
            nc.sync.dma_start(w_t[:], W_in[:])
            a8_t = constp.tile([HC, 8], dt.float32)
            nc.sync.dma_start(a8_t[:], A8_in[:])
            dstrel_t = constp.tile([128, NTILES], dt.int8)
            nc.sync.dma_start(dstrel_t[:], dstrel_in[:])
            if with_bias:
                bias_t = constp.tile([P, HC], dt.float32)
                nc.sync.dma_start(bias_t[:], bias_in[:])

            # rhs_all = [W | W@A8] in bf16
            wT_ps = psp.tile([HC, FIN], dt.float32, space="PSUM", tag="misc", bufs=1)
            nc.tensor.transpose(wT_ps[:], w_t[:], ident[:])
            wT_sb = nap.tile([HC, FIN], dt.float32, tag="wt")
            nc.vector.tensor_copy(wT_sb[:], wT_ps[:])
            wsc_ps = psp.tile([FIN, 8], dt.float32, space="PSUM", tag="misc", bufs=1)
            nc.tensor.matmul(
                wsc_ps[:], lhsT=wT_sb[:], rhs=a8_t[:], start=True, stop=True
            )
            rhs_all = nap.tile([FIN, HC + 8], dt.bfloat16, tag="rhsall", bufs=1)
            nc.vector.tensor_copy(rhs_all[:, 0:HC], w_t[:])
            nc.vector.tensor_copy(rhs_all[:, HC : HC + 8], wsc_ps[:])

            nc.sync.dma_start(shard[NPC:SHARD_ROWS, :], dummy_in[0:4])

            # persistent a_dst table: [128, BLOCKS*4] bf16
            adst_sb = constp.tile([P, BLOCKS * 4], dt.bfloat16)

            # ---------- phase A + pipelined AllGather quarters ----------
            for t in range(BLOCKS):
                xt = nap.tile([FIN, P], dt.bfloat16, tag="xt")
                nc.sync.dma_start(xt[:], xT_in[t])
                aps = psp.tile(
                    [P, HC + 8], dt.float32, space="PSUM", tag="aps", bufs=2
                )
                nc.tensor.matmul(
                    aps[:], lhsT=xt[:], rhs=rhs_all[:], start=True, stop=True
                )
                rec = nap.tile([P, REC_F32], dt.float32, tag="rec")
                rec8 = rec[:].bitcast(dt.float8e4)
                nc.vector.tensor_copy(rec8[:, 0:HC], aps[:, 0:HC])
                nc.scalar.activation(
                    rec[:, 32:36], aps[:, HC : HC + 4], act.Copy
                )
                nc.scalar.activation(
                    adst_sb[:, t * 4 : (t + 1) * 4], aps[:, HC + 4 : HC + 8], act.Copy
                )
                nc.sync.dma_start(shard[t * P : (t + 1) * P, :], rec[:, :])

            # ---------- phase B ----------
            nc.gpsimd.collective_compute(
                "AllGather",
                alu.bypass,
                replica_groups=[list(range(NCORES))],
                ins=[shard[:, :]],
                outs=[table[:, :]],
            )

            # ---------- phase C ----------
            pooled_ps = psp.tile([B, HC], dt.float32, space="PSUM", tag="pool", bufs=1)

            # software pipeline state
            pend = {}  # b -> dict(acc, goh, srec, srec2, outb, outb2, outbf)
            prep = {}  # b -> (oh, adst_ps)

            def issue_drep(b):
                drep = ohp.tile([P, T_BLK, P], dt.int8, tag="drep", bufs=3)
                nc.sync.dma_start(
                    drep[:].rearrange("p t q -> p (t q)"),
                    drow_in[b].to_broadcast((P, T_BLK * P)),
                )
                return drep

            def build_prep_dve(b, drep):
                ohT = ohp.tile([P, T_BLK, P], dt.bfloat16, tag="ohT", bufs=2)
                nc.vector.tensor_tensor(
                    out=ohT[:],
                    in0=drep[:],
                    in1=iotac[:]
                    .rearrange("p x -> p x ()")
                    .to_broadcast((P, T_BLK, P)),
                    op=alu.is_equal,
                )
                oh = ohp.tile([P, T_BLK, P], dt.bfloat16, tag="oh", bufs=2)
                nc.vector.tensor_tensor(
                    out=oh[:],
                    in0=dstrel_t[:, b * T_BLK : (b + 1) * T_BLK]
                    .rearrange("p t -> p t ()")
                    .to_broadcast((P, T_BLK, P)),
                    in1=iota[:].to_broadcast((P, T_BLK, P)),
                    op=alu.is_equal,
                )
                prep[b] = (oh, ohT, None)

            def build_prep_pe(b):
                oh, ohT, _ = prep[b]
                adst_ps = psp.tile(
                    [P, T_BLK, 4], dt.float32, space="PSUM", tag="adst", bufs=2
                )
                for t in range(T_BLK):
                    nc.tensor.matmul(
                        adst_ps[:, t, :],
                        lhsT=ohT[:, t, :],
                        rhs=adst_sb[:, b * 4 : (b + 1) * 4],
                        start=True,
                        stop=True,
                    )
                prep[b] = (oh, ohT, adst_ps)

            def flush_dve1(b):
                d = pend[b]
                srec = flp.tile([P, 4], dt.float32, tag="srec")
                nc.vector.tensor_scalar(
                    out=srec[:],
                    in0=d["acc"][:, HC : HC + 4],
                    scalar1=1e-30,
                    scalar2=None,
                    op0=alu.max,
                )
                nc.vector.reciprocal(srec[:], srec[:])
                d["srec"] = srec

            def flush_scalar(b):
                d = pend[b]
                if with_bias:
                    return
                outb = flp.tile([P, HC], dt.float32, tag="outb")
                outb2 = flp.tile([P, HC], dt.float32, tag="outb2")
                for h in range(H):
                    nc.scalar.activation(
                        outb[:, h * C : (h + 1) * C],
                        d["acc"][:, h * C : (h + 1) * C],
                        act.Copy,
                        scale=d["srec"][:, h : h + 1],
                    )
                nc.scalar.activation(outb2[:], outb[:], act.Copy, scale=NEG_ACT)
                d["outb"], d["outb2"] = outb, outb2

            def flush_dve2(b):
                d = pend[b]
                outbf = flp.tile([P, HC], dt.bfloat16, tag="outbf")
                if with_bias:
                    outb = flp.tile([P, HC], dt.float32, tag="outb")
                    nc.vector.tensor_tensor(
                        out=outb[:].rearrange("p (h c) -> p h c", h=H),
                        in0=d["acc"][:, 0:HC].rearrange("p (h c) -> p h c", h=H),
                        in1=d["srec"][:]
                        .rearrange("p h -> p h ()")
                        .to_broadcast((P, H, C)),
                        op=alu.mult,
                    )
                    nc.vector.tensor_add(outb[:], outb[:], bias_t[:])
                    tmpo = flp.tile([P, HC], dt.float32, tag="tmpo")
                    nc.vector.tensor_scalar_mul(tmpo[:], outb[:], NEG_ACT)
                    nc.vector.tensor_tensor(
                        out=outbf[:], in0=outb[:], in1=tmpo[:], op=alu.max
                    )
                else:
                    nc.vector.tensor_tensor(
                        out=outbf[:], in0=d["outb"][:], in1=d["outb2"][:], op=alu.max
                    )
                d["outbf"] = outbf

            def flush_pool(b):
                d = pend.pop(b)
                nc.tensor.matmul(
                    pooled_ps[:],
                    lhsT=d["goh"][:],
                    rhs=d["outbf"][:],
                    start=(b == 0),
                    stop=(b == BLOCKS - 1),
                )

            # prologue: drep(0), drep(1), prep(0)
            dreps = {0: issue_drep(0), 1: issue_drep(1)}
            build_prep_dve(0, dreps.pop(0))
            build_prep_pe(0)
            for s in range(NSB):
                gwin = gwp.tile(
                    [P, NCHUNK, WIN_T, REC_F32], dt.float32, tag="gwin", bufs=3
                )
                for c in range(NCHUNK):
                    off = (s * NCHUNK + c) * WIN_MAIN
                    ix1 = ixp.tile([128, WIN_MAIN // 16], dt.int16, tag="ix1", bufs=8)
                    nc.sync.dma_start(
                        ix1[:], isrc_in[:, off // 16 : (off + WIN_MAIN) // 16]
                    )
                    nc.gpsimd.dma_gather(
                        out_ap=gwin[:, c, :, :],
                        in_ap=table[c * CHUNK_ROWS : (c + 1) * CHUNK_ROWS, :],
                        idxs_ap=ix1[:],
                        num_idxs=WIN_MAIN,
                        num_idxs_reg=WIN_MAIN,
                        elem_size=REC_F32,
                        single_packet=False,
                        queue_num=c % NQ,
                    )

                for bb in range(SB):
                    b = s * SB + bb
                    sl = slice(bb * t_bc, (bb + 1) * t_bc)
                    oh, _ohT, adst_ps = prep.pop(b)
                    pblk = edp.tile([P, T_BLK, 4], dt.float32, tag="pblk")
                    nc.vector.tensor_tensor(
                        out=pblk[:].rearrange("p (c t) h -> p c t h", c=NCHUNK),
                        in0=gwin[:, :, sl, 32:36],
                        in1=adst_ps[:].rearrange("p (c t) h -> p c t h", c=NCHUNK),
                        op=alu.add,
                    )
                    # exp(leaky_s(x)) == max(exp(x), exp(s*x)) for 0<s<1
                    pexp1 = edp.tile([P, T_BLK, 4], dt.float32, tag="pexp1")
                    nc.scalar.activation(pexp1[:], pblk[:], act.Exp)
                    pexp2 = edp.tile([P, T_BLK, 4], dt.float32, tag="pexp2")
                    nc.scalar.activation(pexp2[:], pblk[:], act.Exp, scale=NEG_ATT)
                    # DVE fillers while the scalar exps run
                    if b > 0:
                        flush_dve1(b - 1)
                    if b + 2 < BLOCKS:
                        dreps[b + 2] = issue_drep(b + 2)
                    if b + 1 < BLOCKS:
                        build_prep_dve(b + 1, dreps.pop(b + 1))
                    pbf = edp.tile([P, T_BLK, 4], dt.bfloat16, tag="pbf")
                    nc.vector.tensor_tensor(
                        out=pbf[:], in0=pexp1[:], in1=pexp2[:], op=alu.max
                    )

                    acc = psp.tile(
                        [P, HC + 4], dt.float32, space="PSUM", tag="acc", bufs=2
                    )
                    msg = edp.tile(
                        [P, T_BLK, HC + 4], dt.bfloat16, tag="msg", bufs=3
                    )
                    nc.vector.tensor_copy(msg[:, :, HC : HC + 4], pbf[:])
                    gb = gwin[:, :, sl, :].bitcast(dt.float8e4)
                    for c in range(NCHUNK):
                        slp = slice(c * t_bc, (c + 1) * t_bc)
                        nc.vector.tensor_tensor(
                            out=msg[:, slp, 0:HC].rearrange(
                                "p t (h c2) -> p t h c2", h=H
                            ),
                            in0=gb[:, c, :, 0:HC].rearrange(
                                "p t (h c2) -> p t h c2", h=H
                            ),
                            in1=pbf[:, slp, :]
                            .rearrange("p t h -> p t h ()")
                            .to_broadcast((P, t_bc, H, C)),
                            op=alu.mult,
                        )
                    for t_in_blk in range(T_BLK):
                        nc.tensor.matmul(
                            acc[:],
                            lhsT=oh[:, t_in_blk, :],
                            rhs=msg[:, t_in_blk, :],
                            start=(t_in_blk == 0),
                            stop=(t_in_blk == T_BLK - 1),
                        )
                    if b + 1 < BLOCKS:
                        build_prep_pe(b + 1)
                    goh = flp.tile([P, B], dt.bfloat16, tag="goh")
                    nc.sync.dma_start(goh[:], goh_in[b])
                    pend[b] = {"acc": acc, "goh": goh}
                    if b > 0:
                        flush_scalar(b - 1)
                        flush_dve2(b - 1)
                        flush_pool(b - 1)
            flush_dve1(BLOCKS - 1)
            flush_scalar(BLOCKS - 1)
            flush_dve2(BLOCKS - 1)
            flush_pool(BLOCKS - 1)

            # ---------- pooling + final linear ----------
            pooled_sb = nap.tile([B, HC], dt.float32, tag="poolsb", bufs=1)
            nc.vector.tensor_copy(pooled_sb[:], pooled_ps[:])
            nc.sync.dma_start(pool_in[:], pooled_sb[:])
            nc.gpsimd.collective_compute(
                "AllReduce",
                alu.add,
                replica_groups=[list(range(NCORES))],
                ins=[pool_in[:]],
                outs=[pool_out[:]],
            )
            pooled2 = nap.tile([B, HC], dt.float32, tag="pool2", bufs=1)
            nc.sync.dma_start(pooled2[:], pool_out[:])
            crecip = nap.tile([B, 1], dt.float32, tag="crecip", bufs=1)
            nc.sync.dma_start(crecip[:], crecip_in[:])
            nc.vector.tensor_scalar(
                out=pooled2[:],
                in0=pooled2[:],
                scalar1=crecip[:],
                scalar2=None,
                op0=alu.mult,
            )
            p2T_ps = psp.tile([HC, B], dt.float32, space="PSUM", tag="misc", bufs=1)
            nc.tensor.transpose(p2T_ps[:], pooled2[:], ident[:])
            p2T = nap.tile([HC, B], dt.float32, tag="p2T", bufs=1)
            nc.vector.tensor_copy(p2T[:], p2T_ps[:])
            linw_t = nap.tile([HC, NCLS], dt.float32, tag="linw", bufs=1)
            nc.sync.dma_start(linw_t[:], linw_in[:])
            fin_ps = psp.tile([B, NCLS], dt.float32, space="PSUM", tag="misc", bufs=1)
            nc.tensor.matmul(
                fin_ps[:], lhsT=p2T[:], rhs=linw_t[:], start=True, stop=True
            )
            fin_sb = nap.tile([B, NCLS], dt.float32, tag="finsb", bufs=1)
            nc.vector.tensor_copy(fin_sb[:], fin_ps[:])
            linb_t = nap.tile([B, NCLS], dt.float32, tag="linb", bufs=1)
            nc.sync.dma_start(linb_t[:], linb_in[:])
            nc.vector.tensor_add(fin_sb[:], fin_sb[:], linb_t[:])
            nc.sync.dma_start(out_fin[:], fin_sb[:])

    nc.compile()
    return nc


def _in_maps(hp):
    maps = []
    for k in range(NCORES):
        maps.append(
            {
                "xT_in": hp["xT_tiles"][k],
                "W_in": hp["W"],
                "A8_in": hp["A8"],
                "ident_in": hp["identity"],
                "iota_in": hp["iota_bf"],
                "iotac_in": hp["iota_col"],
                "bias_in": hp["bias_rep"],
                "dummy_in": hp["dummy_row"],
                "isrc_in": hp["idx_src_w"][k],
                "dstrel_in": hp["dstrel_w"][k],
                "drow_in": hp["dstrel_row"][k],
                "goh_in": hp["g_onehot"][k],
                "crecip_in": hp["counts_recip"],
                "linw_in": hp["linwT"],
                "linb_in": hp["linb_rep"],
            }
        )
    return maps


def kernel(x, edge_index, batch, batch_size, W, att_src, att_dst, bias, lin_w, lin_b):
    hp = _build_host_data(x, edge_index, batch, W, att_src, att_dst, bias, lin_w, lin_b)
    nc = _build_nc(hp)
    from concourse.bass_utils import run_bass_kernel_spmd

    res = run_bass_kernel_spmd(
        nc, _in_maps(hp), core_ids=list(range(NCORES)), trace=False
    )
    return np.asarray(res.results[0]["out_fin"], dtype=np.float32)


# revision 48
# speedup vs baseline: 1.0422x; 1.0296x over previous
"""Distributed GAT (GATConv eval + global mean pool + linear) on 8 TRN2 NeuronCores.

Pipeline (shapes hardcoded for nn_GAT_27968827032308):
  Host: renumber nodes -> 8 cores x 100 blocks x 128 slots (degree-balanced deal
    packing); route each edge (incl. self-loops) to the core owning its dst;
    order per-core edges [block, src-chunk], pad runs to 128-slot tiles; build
    int16 gather-index stream plus per-tile dst-onehot scalar streams.
  Phase A (device, per core): records for own nodes: xh = x@W (PE, bf16),
    a_src/a_dst = x@(W@[Asrc|Adst]); 512B record rows
    [xh bf16 x128 | a_src f32 x4 | pad] -> DRAM shard; a_dst kept in a
    persistent SBUF table [128, BLOCKS*4] bf16 (dst side is always local).
  Phase B: AllGather shards -> replicated full record table.
  Phase C (per 5-block superblock): one dma_gather per src-chunk pulls
    25 record tiles; per block: onehot oh[slot,node] and transposed onehot
    ohT[node,slot] built in single batched DVE ops; a_dst delivered to edge
    slots via per-tile PE matmuls ohT^T @ adst_blk (no second gather);
    p = exp(leaky_0.2(a_src+a_dst)) on the Scalar engine; batched 4D msg
    multiply; PE matmul acc[n,0:128] += oh^T @ (p*xh), acc[n,128:132] += oh^T@p
    accumulated in PSUM per node-block; flush via Scalar Lrelu(acc*1/s).
  Pooling: PE matmul with host-built graph-onehot -> partial pooled sums
    [128 graphs, 128]; AllReduce; * (1/count); final linear via PE.

kernel(**inputs): FULL inputs -> FULL [128, 10] float32 output.
"""

import sys

sys.path.insert(0, "/opt/trn_rl_repo")
sys.path.insert(0, "/opt/trn_rl_repo/concourse")

import numpy as np

N = 100000
FIN = 128
H = 4
C = 32
HC = H * C
B = 128
NCLS = 10
NEG_ATT = 0.2
NEG_ACT = 0.01

NCORES = 8
BLOCKS = 100
P = 128
NPC = BLOCKS * P  # 12800 padded node slots per core
T_BC_MIN = 5
NCHUNK = 4
SHARD_ROWS = NPC + 4  # + pad rows; dummy record at SHARD_ROWS-1
CHUNK_ROWS = 2 * SHARD_ROWS  # 25608 < 32768 (int16 gather idx)
TBL_ROWS = NCORES * SHARD_ROWS
DUMMY_REL = SHARD_ROWS - 1  # same rel for every chunk (core 2c's dummy row)
REC_F32 = 64  # 256B records: [xh fp8e4m3 x128 | a_src f32 x4 | pad]
SB = 5  # blocks per gather superblock
SCRATCH = 32768
NQ = 4


def _pack_nodes(deg):
    """Snake-deal nodes (by degree desc) into NCORES*BLOCKS bins, <=128 each."""
    nbins = NCORES * BLOCKS
    order = np.argsort(-deg, kind="stable")
    nrounds = (N + nbins - 1) // nbins
    bin_of = np.empty(N, dtype=np.int64)
    slot_of = np.empty(N, dtype=np.int64)
    for r in range(nrounds):
        seg = order[r * nbins : (r + 1) * nbins]
        idx = np.arange(len(seg))
        b = idx if r % 2 == 0 else (nbins - 1 - idx)
        bin_of[seg] = b
        slot_of[seg] = r
    assert slot_of.max() < P
    return bin_of, slot_of


def _build_host_data(x, edge_index, batch, W, att_src, att_dst, bias, lin_w, lin_b):
    import ml_dtypes

    x = np.asarray(x, dtype=np.float32)
    ei = np.asarray(edge_index, dtype=np.int64)
    batch = np.asarray(batch, dtype=np.int64)
    W = np.ascontiguousarray(np.asarray(W, dtype=np.float32))
    att_src = np.asarray(att_src, dtype=np.float32)
    att_dst = np.asarray(att_dst, dtype=np.float32)
    bias = np.asarray(bias, dtype=np.float32)
    lin_w = np.asarray(lin_w, dtype=np.float32)
    lin_b = np.asarray(lin_b, dtype=np.float32)

    src_all = np.concatenate([ei[0], np.arange(N, dtype=np.int64)])
    dst_all = np.concatenate([ei[1], np.arange(N, dtype=np.int64)])
    E_tot = src_all.shape[0]

    deg_in = np.bincount(dst_all, minlength=N)
    bin_of, slot_of = _pack_nodes(deg_in)
    core_of = bin_of // BLOCKS
    block_of = bin_of % BLOCKS
    pos_of = block_of * P + slot_of  # [N] position within core shard

    e_core = core_of[dst_all]
    e_block = block_of[dst_all]
    e_chunk = core_of[src_all] // 2

    cnt = np.zeros((NCORES, BLOCKS, NCHUNK), dtype=np.int64)
    np.add.at(cnt, (e_core, e_block, e_chunk), 1)
    cap = T_BC_MIN * P
    # chunk-aware fixup: move nodes out of bins whose per-chunk load exceeds cap
    if cnt.max() > cap:
        nbins = NCORES * BLOCKS
        node_chunk_cnt = np.zeros((N, NCHUNK), dtype=np.int64)
        np.add.at(node_chunk_cnt, (dst_all, e_chunk), 1)
        bin_cnt = cnt.reshape(nbins, NCHUNK).copy()
        bin_fill = np.bincount(bin_of, minlength=nbins)
        nodes_by_bin = [list(np.where(bin_of == bb)[0]) for bb in range(nbins)]
        # same core-pair only: keeps every src's chunk assignment unchanged
        qgrp_of_bin = np.arange(nbins) // (2 * BLOCKS)
        for _ in range(200000):
            worst = int(np.argmax(bin_cnt.max(axis=1)))
            if bin_cnt[worst].max() <= cap:
                break
            cands = nodes_by_bin[worst]
            ci = int(np.argmax(bin_cnt[worst]))
            best_n = max(cands, key=lambda n: node_chunk_cnt[n, ci])
            headroom = cap - bin_cnt - node_chunk_cnt[best_n][None, :]
            ok = (
                (headroom.min(axis=1) >= 0)
                & (bin_fill < P)
                & (qgrp_of_bin == qgrp_of_bin[worst])
            )
            ok[worst] = False
            if not ok.any():
                break
            tgt = int(np.argmax(np.where(ok, headroom.min(axis=1), -(10**9))))
            nodes_by_bin[worst].remove(best_n)
            nodes_by_bin[tgt].append(best_n)
            bin_cnt[worst] -= node_chunk_cnt[best_n]
            bin_cnt[tgt] += node_chunk_cnt[best_n]
            bin_fill[worst] -= 1
            bin_fill[tgt] += 1
            bin_of[best_n] = tgt
        # recompute placement-dependent arrays
        for bb in range(nbins):
            ns = nodes_by_bin[bb]
            slot_of[ns] = np.arange(len(ns))
        core_of = bin_of // BLOCKS
        block_of = bin_of % BLOCKS
        pos_of = block_of * P + slot_of
        e_core = core_of[dst_all]
        e_block = block_of[dst_all]
        e_chunk = core_of[src_all] // 2
        cnt = np.zeros((NCORES, BLOCKS, NCHUNK), dtype=np.int64)
        np.add.at(cnt, (e_core, e_block, e_chunk), 1)
    t_bc = max(int(np.ceil(cnt.max() / P)), T_BC_MIN)
    T_BLK = t_bc * NCHUNK
    NTILES = BLOCKS * T_BLK
    SLOTS = NTILES * P
    RUN = t_bc * P

    run_base = (e_block * NCHUNK + e_chunk) * RUN
    key = (e_core * BLOCKS + e_block) * NCHUNK + e_chunk
    order = np.argsort(key, kind="stable")
    ks = key[order]
    run_start = np.searchsorted(ks, np.arange(NCORES * BLOCKS * NCHUNK))
    within = np.empty(E_tot, dtype=np.int64)
    within[order] = np.arange(E_tot) - run_start[ks]
    slot = run_base + within

    idx_src16 = np.full((NCORES, SLOTS), DUMMY_REL, dtype=np.int16)
    dstrel = np.full((NCORES, SLOTS), -7, dtype=np.int8)
    tblrow = core_of[src_all] * SHARD_ROWS + pos_of[src_all]
    idx_src16[e_core, slot] = (tblrow - e_chunk * CHUNK_ROWS).astype(np.int16)
    dstrel[e_core, slot] = (pos_of[dst_all] % P).astype(np.int8)

    # gather-window slot order: (sb, chunk) -> SB consecutive blocks' chunk runs
    NSB = BLOCKS // SB
    WIN_MAIN = SB * RUN
    win_flat = np.empty(NSB * NCHUNK * WIN_MAIN, dtype=np.int64)
    w = 0
    for s in range(NSB):
        for c in range(NCHUNK):
            for b in range(SB * s, SB * s + SB):
                base = (b * NCHUNK + c) * RUN
                win_flat[w : w + RUN] = np.arange(base, base + RUN)
                w += RUN
    assert w == SLOTS

    def wrap16(vals):
        n = vals.shape[0]
        wv = vals.reshape(n // 16, 16).T.astype(np.int16)
        return np.tile(wv, (8, 1))

    idx_src_w = np.zeros((NCORES, 128, SLOTS // 16), dtype=np.int16)
    dstrel_w = np.zeros((NCORES, 128, NTILES), dtype=np.int8)
    dstrel_row = np.zeros((NCORES, BLOCKS, 1, T_BLK * P), dtype=np.int8)
    for k in range(NCORES):
        idx_src_w[k] = wrap16(idx_src16[k][win_flat])
        dk = dstrel[k].reshape(NTILES, P)
        dstrel_w[k] = dk.T
        dstrel_row[k] = dk.reshape(BLOCKS, 1, T_BLK * P)

    # phase A inputs
    allpos = core_of * NPC + pos_of
    xk_flat = np.zeros(NCORES * NPC, dtype=np.int64)
    xk_flat[allpos] = np.arange(N)
    mask_flat = np.zeros(NCORES * NPC, dtype=bool)
    mask_flat[allpos] = True
    xk = xk_flat.reshape(NCORES, NPC)
    nodemask = mask_flat.reshape(NCORES, NPC)
    xT_tiles = np.zeros((NCORES, BLOCKS, FIN, P), dtype=ml_dtypes.bfloat16)
    for k in range(NCORES):
        xs = np.where(nodemask[k][:, None], x[xk[k]], 0.0)
        xT_tiles[k] = np.ascontiguousarray(
            xs.reshape(BLOCKS, P, FIN).transpose(0, 2, 1)
        ).astype(ml_dtypes.bfloat16)

    A8 = np.zeros((HC, 8), dtype=np.float32)
    for h in range(H):
        A8[h * C : (h + 1) * C, h] = att_src[h]
        A8[h * C : (h + 1) * C, 4 + h] = att_dst[h]

    graph_flat = np.zeros(NCORES * NPC, dtype=np.int64)
    graph_flat[allpos] = batch
    gf = graph_flat.reshape(NCORES, NPC)
    g_onehot = np.zeros((NCORES, BLOCKS, P, B), dtype=ml_dtypes.bfloat16)
    for k in range(NCORES):
        oh = (gf[k][:, None] == np.arange(B)[None, :]) & nodemask[k][:, None]
        g_onehot[k] = oh.reshape(BLOCKS, P, B).astype(ml_dtypes.bfloat16)

    counts = np.bincount(batch, minlength=B).astype(np.float32)
    counts_recip = (1.0 / np.maximum(counts, 1.0)).reshape(B, 1)

    dummy_row = np.zeros((4, REC_F32), dtype=np.float32)
    dummy_row[:, 32:36] = -1e30  # a_src of pad rows -> exp() == 0

    iota_bf = np.tile(np.arange(P), (P, 1)).reshape(P, 1, P)

    return dict(
        t_bc=t_bc,
        T_BLK=T_BLK,
        NTILES=NTILES,
        SLOTS=SLOTS,
        NSB=NSB,
        WIN_MAIN=WIN_MAIN,
        idx_src_w=idx_src_w,
        dstrel_w=dstrel_w,
        dstrel_row=dstrel_row,
        xT_tiles=xT_tiles,
        A8=A8,
        g_onehot=g_onehot,
        counts_recip=counts_recip,
        iota_bf=iota_bf.astype(np.int8),
        iota_col=np.arange(P, dtype=np.int8).reshape(P, 1),
        bias_np=bias,
        bias_rep=np.tile(bias[None, :], (P, 1)).astype(np.float32),
        linb_rep=np.tile(lin_b[None, :], (B, 1)).astype(np.float32),
        identity=np.eye(P, dtype=np.float32),
        dummy_row=dummy_row,
        W=W,
        linwT=np.ascontiguousarray(lin_w.T),
    )


def _build_nc(hp):
    import concourse.bacc as bacc
    import concourse.bass as bass
    import concourse.mybir as mybir
    import concourse.tile as tile

    t_bc = hp["t_bc"]
    T_BLK = hp["T_BLK"]
    NTILES = hp["NTILES"]
    SLOTS = hp["SLOTS"]
    NSB = hp["NSB"]
    WIN_MAIN = hp["WIN_MAIN"]
    WIN_T = SB * t_bc
    with_bias = bool(np.any(hp["bias_np"]))
    dt = mybir.dt
    alu = mybir.AluOpType
    act = mybir.ActivationFunctionType

    nc = bacc.Bacc(
        None,
        target_bir_lowering=False,
        debug=False,
        num_swdge_queues=NQ,
        dynamic_dma_scratch_size=SCRATCH,
        num_devices=NCORES,
    )

    xT_in = nc.dram_tensor("xT_in", [BLOCKS, FIN, P], dt.bfloat16, kind="ExternalInput")
    W_in = nc.dram_tensor("W_in", [FIN, HC], dt.float32, kind="ExternalInput")
    A8_in = nc.dram_tensor("A8_in", [HC, 8], dt.float32, kind="ExternalInput")
    ident_in = nc.dram_tensor("ident_in", [P, P], dt.float32, kind="ExternalInput")
    iota_in = nc.dram_tensor("iota_in", [P, 1, P], dt.int8, kind="ExternalInput")
    iotac_in = nc.dram_tensor("iotac_in", [P, 1], dt.int8, kind="ExternalInput")
    bias_in = nc.dram_tensor("bias_in", [P, HC], dt.float32, kind="ExternalInput")
    dummy_in = nc.dram_tensor(
        "dummy_in", [4, REC_F32], dt.float32, kind="ExternalInput"
    )
    isrc_in = nc.dram_tensor(
        "isrc_in", [128, SLOTS // 16], dt.int16, kind="ExternalInput"
    )
    dstrel_in = nc.dram_tensor(
        "dstrel_in", [128, NTILES], dt.int8, kind="ExternalInput"
    )
    drow_in = nc.dram_tensor(
        "drow_in", [BLOCKS, 1, T_BLK * P], dt.int8, kind="ExternalInput"
    )
    goh_in = nc.dram_tensor("goh_in", [BLOCKS, P, B], dt.bfloat16, kind="ExternalInput")
    crecip_in = nc.dram_tensor("crecip_in", [B, 1], dt.float32, kind="ExternalInput")
    linw_in = nc.dram_tensor("linw_in", [HC, NCLS], dt.float32, kind="ExternalInput")
    linb_in = nc.dram_tensor("linb_in", [B, NCLS], dt.float32, kind="ExternalInput")
    out_fin = nc.dram_tensor("out_fin", [B, NCLS], dt.float32, kind="ExternalOutput")

    shard = nc.dram_tensor(
        "shard", [SHARD_ROWS, REC_F32], dt.float32, kind="Internal"
    )
    table = nc.dram_tensor(
        "table", [TBL_ROWS, REC_F32], dt.float32, kind="Internal", addr_space="Shared"
    )
    pool_in = nc.dram_tensor("pool_in", [B, HC], dt.float32, kind="Internal")
    pool_out = nc.dram_tensor(
        "pool_out", [B, HC], dt.float32, kind="Internal", addr_space="Shared"
    )

    with tile.TileContext(nc) as tc:
        with (
            tc.tile_pool(name="const", bufs=1) as constp,
            tc.tile_pool(name="na", bufs=3) as nap,
            tc.tile_pool(name="ps", bufs=1, space="PSUM") as psp,
            tc.tile_pool(name="gw", bufs=1) as gwp,
            tc.tile_pool(name="ix", bufs=1) as ixp,
            tc.tile_pool(name="oh", bufs=1) as ohp,
            tc.tile_pool(name="ed", bufs=2) as edp,
            tc.tile_pool(name="fl", bufs=2) as flp,
        ):
            iota = constp.tile([P, 1, P], dt.int8)
            nc.sync.dma_start(iota[:], iota_in[:])
            iotac = constp.tile([P, 1], dt.int8)
            nc.sync.dma_start(iotac[:], iotac_in[:])
            ident = constp.tile([P, P], dt.float32)
            nc.sync.dma_start(ident[:], ident_in[:])
            w_t = constp.tile([FIN, HC], dt.float32)
            nc.sync.dma_start(w_t[:], W_in[:])
            a8_t = constp.tile([HC, 8], dt.float32)
            nc.sync.dma_start(a8_t[:], A8_in[:])
            dstrel_t = constp.tile([128, NTILES], dt.int8)
            nc.sync.dma_start(dstrel_t[:], dstrel_in[:])
            if with_bias:
                bias_t = constp.tile([P, HC], dt.float32)
                nc.sync.dma_start(bias_t[:], bias_in[:])

            # rhs_all = [W | W@A8] in bf16
            wT_ps = psp.tile([HC, FIN], dt.float32, space="PSUM", tag="misc", bufs=1)
            nc.tensor.transpose(wT_ps[:], w_t[:], ident[:])
            wT_sb = nap.tile([HC, FIN], dt.float32, tag="wt")
            nc.vector.tensor_copy(wT_sb[:], wT_ps[:])
            wsc_ps = psp.tile([FIN, 8], dt.float32, space="PSUM", tag="misc", bufs=1)
            nc.tensor.matmul(
                wsc_ps[:], lhsT=wT_sb[:], rhs=a8_t[:], start=True, stop=True
            )
            rhs_all = nap.tile([FIN, HC + 8], dt.bfloat16, tag="rhsall", bufs=1)
            nc.vector.tensor_copy(rhs_all[:, 0:HC], w_t[:])
            nc.vector.tensor_copy(rhs_all[:, HC : HC + 8], wsc_ps[:])

            nc.sync.dma_start(shard[NPC:SHARD_ROWS, :], dummy_in[0:4])

            # persistent a_dst table: [128, BLOCKS*4] bf16
            adst_sb = constp.tile([P, BLOCKS * 4], dt.bfloat16)

            # ---------- phase A + pipelined AllGather quarters ----------
            for t in range(BLOCKS):
                xt = nap.tile([FIN, P], dt.bfloat16, tag="xt")
                nc.sync.dma_start(xt[:], xT_in[t])
                aps = psp.tile(
                    [P, HC + 8], dt.float32, space="PSUM", tag="aps", bufs=2
                )
                nc.tensor.matmul(
                    aps[:], lhsT=xt[:], rhs=rhs_all[:], start=True, stop=True
                )
                rec = nap.tile([P, REC_F32], dt.float32, tag="rec")
                rec8 = rec[:].bitcast(dt.float8e4)
                nc.vector.tensor_copy(rec8[:, 0:HC], aps[:, 0:HC])
                nc.scalar.activation(
                    rec[:, 32:36], aps[:, HC : HC + 4], act.Copy
                )
                nc.scalar.activation(
                    adst_sb[:, t * 4 : (t + 1) * 4], aps[:, HC + 4 : HC + 8], act.Copy
                )
                nc.sync.dma_start(shard[t * P : (t + 1) * P, :], rec[:, :])

            # ---------- phase B ----------
            nc.gpsimd.collective_compute(
                "AllGather",
                alu.bypass,
                replica_groups=[list(range(NCORES))],
                ins=[shard[:, :]],
                outs=[table[:, :]],
            )

            # ---------- phase C ----------
            pooled_ps = psp.tile([B, HC], dt.float32, space="PSUM", tag="pool", bufs=1)

            # software pipeline state
            pend = {}  # b -> dict(acc, goh, srec, srec2, outb, outb2, outbf)
            prep = {}  # b -> (oh, adst_ps)

            def issue_drep(b):
                drep = ohp.tile([P, T_BLK, P], dt.int8, tag="drep", bufs=3)
                nc.sync.dma_start(
                    drep[:].rearrange("p t q -> p (t q)"),
                    drow_in[b].to_broadcast((P, T_BLK * P)),
                )
                return drep

            def build_prep_dve(b, drep):
                ohT = ohp.tile([P, T_BLK, P], dt.bfloat16, tag="ohT", bufs=2)
                nc.vector.tensor_tensor(
                    out=ohT[:],
                    in0=drep[:],
                    in1=iotac[:]
                    .rearrange("p x -> p x ()")
                    .to_broadcast((P, T_BLK, P)),
                    op=alu.is_equal,
                )
                oh = ohp.tile([P, T_BLK, P], dt.bfloat16, tag="oh", bufs=2)
                nc.vector.tensor_tensor(
                    out=oh[:],
                    in0=dstrel_t[:, b * T_BLK : (b + 1) * T_BLK]
                    .rearrange("p t -> p t ()")
                    .to_broadcast((P, T_BLK, P)),
                    in1=iota[:].to_broadcast((P, T_BLK, P)),
                    op=alu.is_equal,
                )
                prep[b] = (oh, ohT, None)

            def build_prep_pe(b):
                oh, ohT, _ = prep[b]
                adst_ps = psp.tile(
                    [P, T_BLK, 4], dt.float32, space="PSUM", tag="adst", bufs=2
                )
                for t in range(T_BLK):
                    nc.tensor.matmul(
                        adst_ps[:, t, :],
                        lhsT=ohT[:, t, :],
                        rhs=adst_sb[:, b * 4 : (b + 1) * 4],
                        start=True,
                        stop=True,
                    )
                prep[b] = (oh, ohT, adst_ps)

            def flush_dve1(b):
                d = pend[b]
                srec = flp.tile([P, 4], dt.float32, tag="srec")
                nc.vector.tensor_scalar(
                    out=srec[:],
                    in0=d["acc"][:, HC : HC + 4],
                    scalar1=1e-30,
                    scalar2=None,
                    op0=alu.max,
                )
                nc.vector.reciprocal(srec[:], srec[:])
                d["srec"] = srec

            def flush_scalar(b):
                d = pend[b]
                if with_bias:
                    return
                outb = flp.tile([P, HC], dt.float32, tag="outb")
                outb2 = flp.tile([P, HC], dt.float32, tag="outb2")
                for h in range(H):
                    nc.scalar.activation(
                        outb[:, h * C : (h + 1) * C],
                        d["acc"][:, h * C : (h + 1) * C],
                        act.Copy,
                        scale=d["srec"][:, h : h + 1],
                    )
                nc.scalar.activation(outb2[:], outb[:], act.Copy, scale=NEG_ACT)
                d["outb"], d["outb2"] = outb, outb2

            def flush_dve2(b):
                d = pend[b]
                outbf = flp.tile([P, HC], dt.bfloat16, tag="outbf")
                if with_bias:
                    outb = flp.tile([P, HC], dt.float32, tag="outb")
                    nc.vector.tensor_tensor(
                        out=outb[:].rearrange("p (h c) -> p h c", h=H),
                        in0=d["acc"][:, 0:HC].rearrange("p (h c) -> p h c", h=H),
                        in1=d["srec"][:]
                        .rearrange("p h -> p h ()")
                        .to_broadcast((P, H, C)),
                        op=alu.mult,
                    )
                    nc.vector.tensor_add(outb[:], outb[:], bias_t[:])
                    tmpo = flp.tile([P, HC], dt.float32, tag="tmpo")
                    nc.vector.tensor_scalar_mul(tmpo[:], outb[:], NEG_ACT)
                    nc.vector.tensor_tensor(
                        out=outbf[:], in0=outb[:], in1=tmpo[:], op=alu.max
                    )
                else:
                    nc.vector.tensor_tensor(
                        out=outbf[:], in0=d["outb"][:], in1=d["outb2"][:], op=alu.max
                    )
                d["outbf"] = outbf

            def flush_pool(b):
                d = pend.pop(b)
                nc.tensor.matmul(
                    pooled_ps[:],
                    lhsT=d["goh"][:],
                    rhs=d["outbf"][:],
                    start=(b == 0),
                    stop=(b == BLOCKS - 1),
                )

            # prologue: drep(0), drep(1), prep(0)
            dreps = {0: issue_drep(0), 1: issue_drep(1)}
            build_prep_dve(0, dreps.pop(0))
            build_prep_pe(0)
            for s in range(NSB):
                gwin = gwp.tile(
                    [P, NCHUNK, WIN_T, REC_F32], dt.float32, tag="gwin", bufs=4
                )
                for c in range(NCHUNK):
                    off = (s * NCHUNK + c) * WIN_MAIN
                    ix1 = ixp.tile([128, WIN_MAIN // 16], dt.int16, tag="ix1", bufs=8)
                    nc.sync.dma_start(
                        ix1[:], isrc_in[:, off // 16 : (off + WIN_MAIN) // 16]
                    )
                    nc.gpsimd.dma_gather(
                        out_ap=gwin[:, c, :, :],
                        in_ap=table[c * CHUNK_ROWS : (c + 1) * CHUNK_ROWS, :],
                        idxs_ap=ix1[:],
                        num_idxs=WIN_MAIN,
                        num_idxs_reg=WIN_MAIN,
                        elem_size=REC_F32,
                        single_packet=False,
                        queue_num=c % NQ,
                    )

                for bb in range(SB):
                    b = s * SB + bb
                    sl = slice(bb * t_bc, (bb + 1) * t_bc)
                    oh, _ohT, adst_ps = prep.pop(b)
                    pblk = edp.tile([P, T_BLK, 4], dt.float32, tag="pblk")
                    nc.vector.tensor_tensor(
                        out=pblk[:].rearrange("p (c t) h -> p c t h", c=NCHUNK),
                        in0=gwin[:, :, sl, 32:36],
                        in1=adst_ps[:].rearrange("p (c t) h -> p c t h", c=NCHUNK),
                        op=alu.add,
                    )
                    # exp(leaky_s(x)) == max(exp(x), exp(s*x)) for 0<s<1
                    pexp1 = edp.tile([P, T_BLK, 4], dt.float32, tag="pexp1")
                    nc.scalar.activation(pexp1[:], pblk[:], act.Exp)
                    pexp2 = edp.tile([P, T_BLK, 4], dt.float32, tag="pexp2")
                    nc.scalar.activation(pexp2[:], pblk[:], act.Exp, scale=NEG_ATT)
                    # DVE fillers while the scalar exps run
                    if b > 0:
                        flush_dve1(b - 1)
                    if b + 2 < BLOCKS:
                        dreps[b + 2] = issue_drep(b + 2)
                    if b + 1 < BLOCKS:
                        build_prep_dve(b + 1, dreps.pop(b + 1))
                    pbf = edp.tile([P, T_BLK, 4], dt.bfloat16, tag="pbf")
                    nc.vector.tensor_tensor(
                        out=pbf[:], in0=pexp1[:], in1=pexp2[:], op=alu.max
                    )

                    acc = psp.tile(
                        [P, HC + 4], dt.float32, space="PSUM", tag="acc", bufs=2
                    )
                    msg = edp.tile(
                        [P, T_BLK, HC + 4], dt.bfloat16, tag="msg", bufs=3
                    )
                    nc.vector.tensor_copy(msg[:, :, HC : HC + 4], pbf[:])
                    gb = gwin[:, :, sl, :].bitcast(dt.float8e4)
                    for c in range(NCHUNK):
                        slp = slice(c * t_bc, (c + 1) * t_bc)
                        nc.vector.tensor_tensor(
                            out=msg[:, slp, 0:HC].rearrange(
                                "p t (h c2) -> p t h c2", h=H
                            ),
                            in0=gb[:, c, :, 0:HC].rearrange(
                                "p t (h c2) -> p t h c2", h=H
                            ),
                            in1=pbf[:, slp, :]
                            .rearrange("p t h -> p t h ()")
                            .to_broadcast((P, t_bc, H, C)),
                            op=alu.mult,
                        )
                    for t_in_blk in range(T_BLK):
                        nc.tensor.matmul(
                            acc[:],
                            lhsT=oh[:, t_in_blk, :],
                            rhs=msg[:, t_in_blk, :],
                            start=(t_in_blk == 0),
                            stop=(t_in_blk == T_BLK - 1),
                        )
                    if b + 1 < BLOCKS:
                        build_prep_pe(b + 1)
                    goh = flp.tile([P, B], dt.bfloat16, tag="goh")
                    nc.sync.dma_start(goh[:], goh_in[b])
                    pend[b] = {"acc": acc, "goh": goh}
                    if b > 0:
                        flush_scalar(b - 1)
                        flush_dve2(b - 1)
                        flush_pool(b - 1)
            flush_dve1(BLOCKS - 1)
            flush_scalar(BLOCKS - 1)
            flush_dve2(BLOCKS - 1)
            flush_pool(BLOCKS - 1)

            # ---------- pooling + final linear ----------
            pooled_sb = nap.tile([B, HC], dt.float32, tag="poolsb", bufs=1)
            nc.vector.tensor_copy(pooled_sb[:], pooled_ps[:])
            nc.sync.dma_start(pool_in[:], pooled_sb[:])
            nc.gpsimd.collective_compute(
                "AllReduce",
                alu.add,
                replica_groups=[list(range(NCORES))],
                ins=[pool_in[:]],
                outs=[pool_out[:]],
            )
            pooled2 = nap.tile([B, HC], dt.float32, tag="pool2", bufs=1)
            nc.sync.dma_start(pooled2[:], pool_out[:])
            crecip = nap.tile([B, 1], dt.float32, tag="crecip", bufs=1)
            nc.sync.dma_start(crecip[:], crecip_in[:])
            nc.vector.tensor_scalar(
                out=pooled2[:],
                in0=pooled2[:],
                scalar1=crecip[:],
                scalar2=None,
                op0=alu.mult,
            )
            p2T_ps = psp.tile([HC, B], dt.float32, space="PSUM", tag="misc", bufs=1)
            nc.tensor.transpose(p2T_ps[:], pooled2[:], ident[:])
            p2T = nap.tile([HC, B], dt.float32, tag="p2T", bufs=1)
            nc.vector.tensor_copy(p2T[:], p2T_ps[:])
            linw_t = nap.tile([HC, NCLS], dt.float32, tag="linw", bufs=1)
            nc.sync.dma_start(linw_t[:], linw_in[:])
            fin_ps = psp.tile([B, NCLS], dt.float32, space="PSUM", tag="misc", bufs=1)
            nc.tensor.matmul(
                fin_ps[:], lhsT=p2T[:], rhs=linw_t[:], start=True, stop=True
            )
            fin_sb = nap.tile([B, NCLS], dt.float32, tag="finsb", bufs=1)
            nc.vector.tensor_copy(fin_sb[:], fin_ps[:])
            linb_t = nap.tile([B, NCLS], dt.float32, tag="linb", bufs=1)
            nc.sync.dma_start(linb_t[:], linb_in[:])
            nc.vector.tensor_add(fin_sb[:], fin_sb[:], linb_t[:])
            nc.sync.dma_start(out_fin[:], fin_sb[:])

    nc.compile()
    return nc


def _in_maps(hp):
    maps = []
    for k in range(NCORES):
        maps.append(
            {
                "xT_in": hp["xT_tiles"][k],
                "W_in": hp["W"],
                "A8_in": hp["A8"],
                "ident_in": hp["identity"],
                "iota_in": hp["iota_bf"],
                "iotac_in": hp["iota_col"],
                "bias_in": hp["bias_rep"],
                "dummy_in": hp["dummy_row"],
                "isrc_in": hp["idx_src_w"][k],
                "dstrel_in": hp["dstrel_w"][k],
                "drow_in": hp["dstrel_row"][k],
                "goh_in": hp["g_onehot"][k],
                "crecip_in": hp["counts_recip"],
                "linw_in": hp["linwT"],
                "linb_in": hp["linb_rep"],
            }
        )
    return maps


def kernel(x, edge_index, batch, batch_size, W, att_src, att_dst, bias, lin_w, lin_b):
    hp = _build_host_data(x, edge_index, batch, W, att_src, att_dst, bias, lin_w, lin_b)
    nc = _build_nc(hp)
    from concourse.bass_utils import run_bass_kernel_spmd

    res = run_bass_kernel_spmd(
        nc, _in_maps(hp), core_ids=list(range(NCORES)), trace=False
    )
    return np.asarray(res.results[0]["out_fin"], dtype=np.float32)


# revision 49
# speedup vs baseline: 1.0462x; 1.0038x over previous
"""Distributed GAT (GATConv eval + global mean pool + linear) on 8 TRN2 NeuronCores.

Pipeline (shapes hardcoded for nn_GAT_27968827032308):
  Host: renumber nodes -> 8 cores x 100 blocks x 128 slots (degree-balanced deal
    packing); route each edge (incl. self-loops) to the core owning its dst;
    order per-core edges [block, src-chunk], pad runs to 128-slot tiles; build
    int16 gather-index stream plus per-tile dst-onehot scalar streams.
  Phase A (device, per core): records for own nodes: xh = x@W (PE, bf16),
    a_src/a_dst = x@(W@[Asrc|Adst]); 512B record rows
    [xh bf16 x128 | a_src f32 x4 | pad] -> DRAM shard; a_dst kept in a
    persistent SBUF table [128, BLOCKS*4] bf16 (dst side is always local).
  Phase B: AllGather shards -> replicated full record table.
  Phase C (per 5-block superblock): one dma_gather per src-chunk pulls
    25 record tiles; per block: onehot oh[slot,node] and transposed onehot
    ohT[node,slot] built in single batched DVE ops; a_dst delivered to edge
    slots via per-tile PE matmuls ohT^T @ adst_blk (no second gather);
    p = exp(leaky_0.2(a_src+a_dst)) on the Scalar engine; batched 4D msg
    multiply; PE matmul acc[n,0:128] += oh^T @ (p*xh), acc[n,128:132] += oh^T@p
    accumulated in PSUM per node-block; flush via Scalar Lrelu(acc*1/s).
  Pooling: PE matmul with host-built graph-onehot -> partial pooled sums
    [128 graphs, 128]; AllReduce; * (1/count); final linear via PE.

kernel(**inputs): FULL inputs -> FULL [128, 10] float32 output.
"""

import sys

sys.path.insert(0, "/opt/trn_rl_repo")
sys.path.insert(0, "/opt/trn_rl_repo/concourse")

import numpy as np

N = 100000
FIN = 128
H = 4
C = 32
HC = H * C
B = 128
NCLS = 10
NEG_ATT = 0.2
NEG_ACT = 0.01

NCORES = 8
BLOCKS = 100
P = 128
NPC = BLOCKS * P  # 12800 padded node slots per core
T_BC_MIN = 5
NCHUNK = 4
SHARD_ROWS = NPC + 4  # + pad rows; dummy record at SHARD_ROWS-1
CHUNK_ROWS = 2 * SHARD_ROWS  # 25608 < 32768 (int16 gather idx)
TBL_ROWS = NCORES * SHARD_ROWS
DUMMY_REL = SHARD_ROWS - 1  # same rel for every chunk (core 2c's dummy row)
REC_F32 = 64  # 256B records: [xh fp8e4m3 x128 | a_src f32 x4 | pad]
SB = 5  # blocks per gather superblock
SCRATCH = 49152
NQ = 4


def _pack_nodes(deg):
    """Snake-deal nodes (by degree desc) into NCORES*BLOCKS bins, <=128 each."""
    nbins = NCORES * BLOCKS
    order = np.argsort(-deg, kind="stable")
    nrounds = (N + nbins - 1) // nbins
    bin_of = np.empty(N, dtype=np.int64)
    slot_of = np.empty(N, dtype=np.int64)
    for r in range(nrounds):
        seg = order[r * nbins : (r + 1) * nbins]
        idx = np.arange(len(seg))
        b = idx if r % 2 == 0 else (nbins - 1 - idx)
        bin_of[seg] = b
        slot_of[seg] = r
    assert slot_of.max() < P
    return bin_of, slot_of


def _build_host_data(x, edge_index, batch, W, att_src, att_dst, bias, lin_w, lin_b):
    import ml_dtypes

    x = np.asarray(x, dtype=np.float32)
    ei = np.asarray(edge_index, dtype=np.int64)
    batch = np.asarray(batch, dtype=np.int64)
    W = np.ascontiguousarray(np.asarray(W, dtype=np.float32))
    att_src = np.asarray(att_src, dtype=np.float32)
    att_dst = np.asarray(att_dst, dtype=np.float32)
    bias = np.asarray(bias, dtype=np.float32)
    lin_w = np.asarray(lin_w, dtype=np.float32)
    lin_b = np.asarray(lin_b, dtype=np.float32)

    src_all = np.concatenate([ei[0], np.arange(N, dtype=np.int64)])
    dst_all = np.concatenate([ei[1], np.arange(N, dtype=np.int64)])
    E_tot = src_all.shape[0]

    deg_in = np.bincount(dst_all, minlength=N)
    bin_of, slot_of = _pack_nodes(deg_in)
    core_of = bin_of // BLOCKS
    block_of = bin_of % BLOCKS
    pos_of = block_of * P + slot_of  # [N] position within core shard

    e_core = core_of[dst_all]
    e_block = block_of[dst_all]
    e_chunk = core_of[src_all] // 2

    cnt = np.zeros((NCORES, BLOCKS, NCHUNK), dtype=np.int64)
    np.add.at(cnt, (e_core, e_block, e_chunk), 1)
    cap = T_BC_MIN * P
    # chunk-aware fixup: move nodes out of bins whose per-chunk load exceeds cap
    if cnt.max() > cap:
        nbins = NCORES * BLOCKS
        node_chunk_cnt = np.zeros((N, NCHUNK), dtype=np.int64)
        np.add.at(node_chunk_cnt, (dst_all, e_chunk), 1)
        bin_cnt = cnt.reshape(nbins, NCHUNK).copy()
        bin_fill = np.bincount(bin_of, minlength=nbins)
        nodes_by_bin = [list(np.where(bin_of == bb)[0]) for bb in range(nbins)]
        # same core-pair only: keeps every src's chunk assignment unchanged
        qgrp_of_bin = np.arange(nbins) // (2 * BLOCKS)
        for _ in range(200000):
            worst = int(np.argmax(bin_cnt.max(axis=1)))
            if bin_cnt[worst].max() <= cap:
                break
            cands = nodes_by_bin[worst]
            ci = int(np.argmax(bin_cnt[worst]))
            best_n = max(cands, key=lambda n: node_chunk_cnt[n, ci])
            headroom = cap - bin_cnt - node_chunk_cnt[best_n][None, :]
            ok = (
                (headroom.min(axis=1) >= 0)
                & (bin_fill < P)
                & (qgrp_of_bin == qgrp_of_bin[worst])
            )
            ok[worst] = False
            if not ok.any():
                break
            tgt = int(np.argmax(np.where(ok, headroom.min(axis=1), -(10**9))))
            nodes_by_bin[worst].remove(best_n)
            nodes_by_bin[tgt].append(best_n)
            bin_cnt[worst] -= node_chunk_cnt[best_n]
            bin_cnt[tgt] += node_chunk_cnt[best_n]
            bin_fill[worst] -= 1
            bin_fill[tgt] += 1
            bin_of[best_n] = tgt
        # recompute placement-dependent arrays
        for bb in range(nbins):
            ns = nodes_by_bin[bb]
            slot_of[ns] = np.arange(len(ns))
        core_of = bin_of // BLOCKS
        block_of = bin_of % BLOCKS
        pos_of = block_of * P + slot_of
        e_core = core_of[dst_all]
        e_block = block_of[dst_all]
        e_chunk = core_of[src_all] // 2
        cnt = np.zeros((NCORES, BLOCKS, NCHUNK), dtype=np.int64)
        np.add.at(cnt, (e_core, e_block, e_chunk), 1)
    t_bc = max(int(np.ceil(cnt.max() / P)), T_BC_MIN)
    T_BLK = t_bc * NCHUNK
    NTILES = BLOCKS * T_BLK
    SLOTS = NTILES * P
    RUN = t_bc * P

    run_base = (e_block * NCHUNK + e_chunk) * RUN
    key = (e_core * BLOCKS + e_block) * NCHUNK + e_chunk
    order = np.argsort(key, kind="stable")
    ks = key[order]
    run_start = np.searchsorted(ks, np.arange(NCORES * BLOCKS * NCHUNK))
    within = np.empty(E_tot, dtype=np.int64)
    within[order] = np.arange(E_tot) - run_start[ks]
    slot = run_base + within

    idx_src16 = np.full((NCORES, SLOTS), DUMMY_REL, dtype=np.int16)
    dstrel = np.full((NCORES, SLOTS), -7, dtype=np.int8)
    tblrow = core_of[src_all] * SHARD_ROWS + pos_of[src_all]
    idx_src16[e_core, slot] = (tblrow - e_chunk * CHUNK_ROWS).astype(np.int16)
    dstrel[e_core, slot] = (pos_of[dst_all] % P).astype(np.int8)

    # gather-window slot order: (sb, chunk) -> SB consecutive blocks' chunk runs
    NSB = BLOCKS // SB
    WIN_MAIN = SB * RUN
    win_flat = np.empty(NSB * NCHUNK * WIN_MAIN, dtype=np.int64)
    w = 0
    for s in range(NSB):
        for c in range(NCHUNK):
            for b in range(SB * s, SB * s + SB):
                base = (b * NCHUNK + c) * RUN
                win_flat[w : w + RUN] = np.arange(base, base + RUN)
                w += RUN
    assert w == SLOTS

    def wrap16(vals):
        n = vals.shape[0]
        wv = vals.reshape(n // 16, 16).T.astype(np.int16)
        return np.tile(wv, (8, 1))

    idx_src_w = np.zeros((NCORES, 128, SLOTS // 16), dtype=np.int16)
    dstrel_w = np.zeros((NCORES, 128, NTILES), dtype=np.int8)
    dstrel_row = np.zeros((NCORES, BLOCKS, 1, T_BLK * P), dtype=np.int8)
    for k in range(NCORES):
        idx_src_w[k] = wrap16(idx_src16[k][win_flat])
        dk = dstrel[k].reshape(NTILES, P)
        dstrel_w[k] = dk.T
        dstrel_row[k] = dk.reshape(BLOCKS, 1, T_BLK * P)

    # phase A inputs
    allpos = core_of * NPC + pos_of
    xk_flat = np.zeros(NCORES * NPC, dtype=np.int64)
    xk_flat[allpos] = np.arange(N)
    mask_flat = np.zeros(NCORES * NPC, dtype=bool)
    mask_flat[allpos] = True
    xk = xk_flat.reshape(NCORES, NPC)
    nodemask = mask_flat.reshape(NCORES, NPC)
    xT_tiles = np.zeros((NCORES, BLOCKS, FIN, P), dtype=ml_dtypes.bfloat16)
    for k in range(NCORES):
        xs = np.where(nodemask[k][:, None], x[xk[k]], 0.0)
        xT_tiles[k] = np.ascontiguousarray(
            xs.reshape(BLOCKS, P, FIN).transpose(0, 2, 1)
        ).astype(ml_dtypes.bfloat16)

    A8 = np.zeros((HC, 8), dtype=np.float32)
    for h in range(H):
        A8[h * C : (h + 1) * C, h] = att_src[h]
        A8[h * C : (h + 1) * C, 4 + h] = att_dst[h]

    graph_flat = np.zeros(NCORES * NPC, dtype=np.int64)
    graph_flat[allpos] = batch
    gf = graph_flat.reshape(NCORES, NPC)
    g_onehot = np.zeros((NCORES, BLOCKS, P, B), dtype=ml_dtypes.bfloat16)
    for k in range(NCORES):
        oh = (gf[k][:, None] == np.arange(B)[None, :]) & nodemask[k][:, None]
        g_onehot[k] = oh.reshape(BLOCKS, P, B).astype(ml_dtypes.bfloat16)

    counts = np.bincount(batch, minlength=B).astype(np.float32)
    counts_recip = (1.0 / np.maximum(counts, 1.0)).reshape(B, 1)

    dummy_row = np.zeros((4, REC_F32), dtype=np.float32)
    dummy_row[:, 32:36] = -1e30  # a_src of pad rows -> exp() == 0

    iota_bf = np.tile(np.arange(P), (P, 1)).reshape(P, 1, P)

    return dict(
        t_bc=t_bc,
        T_BLK=T_BLK,
        NTILES=NTILES,
        SLOTS=SLOTS,
        NSB=NSB,
        WIN_MAIN=WIN_MAIN,
        idx_src_w=idx_src_w,
        dstrel_w=dstrel_w,
        dstrel_row=dstrel_row,
        xT_tiles=xT_tiles,
        A8=A8,
        g_onehot=g_onehot,
        counts_recip=counts_recip,
        iota_bf=iota_bf.astype(np.int8),
        iota_col=np.arange(P, dtype=np.int8).reshape(P, 1),
        bias_np=bias,
        bias_rep=np.tile(bias[None, :], (P, 1)).astype(np.float32),
        linb_rep=np.tile(lin_b[None, :], (B, 1)).astype(np.float32),
        identity=np.eye(P, dtype=np.float32),
        dummy_row=dummy_row,
        W=W,
        linwT=np.ascontiguousarray(lin_w.T),
    )


def _build_nc(hp):
    import concourse.bacc as bacc
    import concourse.bass as bass
    import concourse.mybir as mybir
    import concourse.tile as tile

    t_bc = hp["t_bc"]
    T_BLK = hp["T_BLK"]
    NTILES = hp["NTILES"]
    SLOTS = hp["SLOTS"]
    NSB = hp["NSB"]
    WIN_MAIN = hp["WIN_MAIN"]
    WIN_T = SB * t_bc
    with_bias = bool(np.any(hp["bias_np"]))
    dt = mybir.dt
    alu = mybir.AluOpType
    act = mybir.ActivationFunctionType

    nc = bacc.Bacc(
        None,
        target_bir_lowering=False,
        debug=False,
        num_swdge_queues=NQ,
        dynamic_dma_scratch_size=SCRATCH,
        num_devices=NCORES,
    )

    xT_in = nc.dram_tensor("xT_in", [BLOCKS, FIN, P], dt.bfloat16, kind="ExternalInput")
    W_in = nc.dram_tensor("W_in", [FIN, HC], dt.float32, kind="ExternalInput")
    A8_in = nc.dram_tensor("A8_in", [HC, 8], dt.float32, kind="ExternalInput")
    ident_in = nc.dram_tensor("ident_in", [P, P], dt.float32, kind="ExternalInput")
    iota_in = nc.dram_tensor("iota_in", [P, 1, P], dt.int8, kind="ExternalInput")
    iotac_in = nc.dram_tensor("iotac_in", [P, 1], dt.int8, kind="ExternalInput")
    bias_in = nc.dram_tensor("bias_in", [P, HC], dt.float32, kind="ExternalInput")
    dummy_in = nc.dram_tensor(
        "dummy_in", [4, REC_F32], dt.float32, kind="ExternalInput"
    )
    isrc_in = nc.dram_tensor(
        "isrc_in", [128, SLOTS // 16], dt.int16, kind="ExternalInput"
    )
    dstrel_in = nc.dram_tensor(
        "dstrel_in", [128, NTILES], dt.int8, kind="ExternalInput"
    )
    drow_in = nc.dram_tensor(
        "drow_in", [BLOCKS, 1, T_BLK * P], dt.int8, kind="ExternalInput"
    )
    goh_in = nc.dram_tensor("goh_in", [BLOCKS, P, B], dt.bfloat16, kind="ExternalInput")
    crecip_in = nc.dram_tensor("crecip_in", [B, 1], dt.float32, kind="ExternalInput")
    linw_in = nc.dram_tensor("linw_in", [HC, NCLS], dt.float32, kind="ExternalInput")
    linb_in = nc.dram_tensor("linb_in", [B, NCLS], dt.float32, kind="ExternalInput")
    out_fin = nc.dram_tensor("out_fin", [B, NCLS], dt.float32, kind="ExternalOutput")

    shard = nc.dram_tensor(
        "shard", [SHARD_ROWS, REC_F32], dt.float32, kind="Internal"
    )
    table = nc.dram_tensor(
        "table", [TBL_ROWS, REC_F32], dt.float32, kind="Internal", addr_space="Shared"
    )
    pool_in = nc.dram_tensor("pool_in", [B, HC], dt.float32, kind="Internal")
    pool_out = nc.dram_tensor(
        "pool_out", [B, HC], dt.float32, kind="Internal", addr_space="Shared"
    )

    with tile.TileContext(nc) as tc:
        with (
            tc.tile_pool(name="const", bufs=1) as constp,
            tc.tile_pool(name="na", bufs=3) as nap,
            tc.tile_pool(name="ps", bufs=1, space="PSUM") as psp,
            tc.tile_pool(name="gw", bufs=1) as gwp,
            tc.tile_pool(name="ix", bufs=1) as ixp,
            tc.tile_pool(name="oh", bufs=1) as ohp,
            tc.tile_pool(name="ed", bufs=2) as edp,
            tc.tile_pool(name="fl", bufs=2) as flp,
        ):
            iota = constp.tile([P, 1, P], dt.int8)
            nc.sync.dma_start(iota[:], iota_in[:])
            iotac = constp.tile([P, 1], dt.int8)
            nc.sync.dma_start(iotac[:], iotac_in[:])
            ident = constp.tile([P, P], dt.float32)
            nc.sync.dma_start(ident[:], ident_in[:])
            w_t = constp.tile([FIN, HC], dt.float32)
            nc.sync.dma_start(w_t[:], W_in[:])
            a8_t = constp.tile([HC, 8], dt.float32)
            nc.sync.dma_start(a8_t[:], A8_in[:])
            dstrel_t = constp.tile([128, NTILES], dt.int8)
            nc.sync.dma_start(dstrel_t[:], dstrel_in[:])
            if with_bias:
                bias_t = constp.tile([P, HC], dt.float32)
                nc.sync.dma_start(bias_t[:], bias_in[:])

            # rhs_all = [W | W@A8] in bf16
            wT_ps = psp.tile([HC, FIN], dt.float32, space="PSUM", tag="misc", bufs=1)
            nc.tensor.transpose(wT_ps[:], w_t[:], ident[:])
            wT_sb = nap.tile([HC, FIN], dt.float32, tag="wt")
            nc.vector.tensor_copy(wT_sb[:], wT_ps[:])
            wsc_ps = psp.tile([FIN, 8], dt.float32, space="PSUM", tag="misc", bufs=1)
            nc.tensor.matmul(
                wsc_ps[:], lhsT=wT_sb[:], rhs=a8_t[:], start=True, stop=True
            )
            rhs_all = nap.tile([FIN, HC + 8], dt.bfloat16, tag="rhsall", bufs=1)
            nc.vector.tensor_copy(rhs_all[:, 0:HC], w_t[:])
            nc.vector.tensor_copy(rhs_all[:, HC : HC + 8], wsc_ps[:])

            nc.sync.dma_start(shard[NPC:SHARD_ROWS, :], dummy_in[0:4])

            # persistent a_dst table: [128, BLOCKS*4] bf16
            adst_sb = constp.tile([P, BLOCKS * 4], dt.bfloat16)

            # ---------- phase A + pipelined AllGather quarters ----------
            for t in range(BLOCKS):
                xt = nap.tile([FIN, P], dt.bfloat16, tag="xt")
                nc.sync.dma_start(xt[:], xT_in[t])
                aps = psp.tile(
                    [P, HC + 8], dt.float32, space="PSUM", tag="aps", bufs=2
                )
                nc.tensor.matmul(
                    aps[:], lhsT=xt[:], rhs=rhs_all[:], start=True, stop=True
                )
                rec = nap.tile([P, REC_F32], dt.float32, tag="rec")
                rec8 = rec[:].bitcast(dt.float8e4)
                nc.vector.tensor_copy(rec8[:, 0:HC], aps[:, 0:HC])
                nc.scalar.activation(
                    rec[:, 32:36], aps[:, HC : HC + 4], act.Copy
                )
                nc.scalar.activation(
                    adst_sb[:, t * 4 : (t + 1) * 4], aps[:, HC + 4 : HC + 8], act.Copy
                )
                nc.sync.dma_start(shard[t * P : (t + 1) * P, :], rec[:, :])

            # ---------- phase B ----------
            nc.gpsimd.collective_compute(
                "AllGather",
                alu.bypass,
                replica_groups=[list(range(NCORES))],
                ins=[shard[:, :]],
                outs=[table[:, :]],
            )

            # ---------- phase C ----------
            pooled_ps = psp.tile([B, HC], dt.float32, space="PSUM", tag="pool", bufs=1)

            # software pipeline state
            pend = {}  # b -> dict(acc, goh, srec, srec2, outb, outb2, outbf)
            prep = {}  # b -> (oh, adst_ps)

            def issue_drep(b):
                drep = ohp.tile([P, T_BLK, P], dt.int8, tag="drep", bufs=3)
                nc.sync.dma_start(
                    drep[:].rearrange("p t q -> p (t q)"),
                    drow_in[b].to_broadcast((P, T_BLK * P)),
                )
                return drep

            def build_prep_dve(b, drep):
                ohT = ohp.tile([P, T_BLK, P], dt.bfloat16, tag="ohT", bufs=2)
                nc.vector.tensor_tensor(
                    out=ohT[:],
                    in0=drep[:],
                    in1=iotac[:]
                    .rearrange("p x -> p x ()")
                    .to_broadcast((P, T_BLK, P)),
                    op=alu.is_equal,
                )
                oh = ohp.tile([P, T_BLK, P], dt.bfloat16, tag="oh", bufs=2)
                nc.vector.tensor_tensor(
                    out=oh[:],
                    in0=dstrel_t[:, b * T_BLK : (b + 1) * T_BLK]
                    .rearrange("p t -> p t ()")
                    .to_broadcast((P, T_BLK, P)),
                    in1=iota[:].to_broadcast((P, T_BLK, P)),
                    op=alu.is_equal,
                )
                prep[b] = (oh, ohT, None)

            def build_prep_pe(b):
                oh, ohT, _ = prep[b]
                adst_ps = psp.tile(
                    [P, T_BLK, 4], dt.float32, space="PSUM", tag="adst", bufs=2
                )
                for t in range(T_BLK):
                    nc.tensor.matmul(
                        adst_ps[:, t, :],
                        lhsT=ohT[:, t, :],
                        rhs=adst_sb[:, b * 4 : (b + 1) * 4],
                        start=True,
                        stop=True,
                    )
                prep[b] = (oh, ohT, adst_ps)

            def flush_dve1(b):
                d = pend[b]
                srec = flp.tile([P, 4], dt.float32, tag="srec")
                nc.vector.tensor_scalar(
                    out=srec[:],
                    in0=d["acc"][:, HC : HC + 4],
                    scalar1=1e-30,
                    scalar2=None,
                    op0=alu.max,
                )
                nc.vector.reciprocal(srec[:], srec[:])
                d["srec"] = srec

            def flush_scalar(b):
                d = pend[b]
                if with_bias:
                    return
                outb = flp.tile([P, HC], dt.float32, tag="outb")
                outb2 = flp.tile([P, HC], dt.float32, tag="outb2")
                for h in range(H):
                    nc.scalar.activation(
                        outb[:, h * C : (h + 1) * C],
                        d["acc"][:, h * C : (h + 1) * C],
                        act.Copy,
                        scale=d["srec"][:, h : h + 1],
                    )
                nc.scalar.activation(outb2[:], outb[:], act.Copy, scale=NEG_ACT)
                d["outb"], d["outb2"] = outb, outb2

            def flush_dve2(b):
                d = pend[b]
                outbf = flp.tile([P, HC], dt.bfloat16, tag="outbf")
                if with_bias:
                    outb = flp.tile([P, HC], dt.float32, tag="outb")
                    nc.vector.tensor_tensor(
                        out=outb[:].rearrange("p (h c) -> p h c", h=H),
                        in0=d["acc"][:, 0:HC].rearrange("p (h c) -> p h c", h=H),
                        in1=d["srec"][:]
                        .rearrange("p h -> p h ()")
                        .to_broadcast((P, H, C)),
                        op=alu.mult,
                    )
                    nc.vector.tensor_add(outb[:], outb[:], bias_t[:])
                    tmpo = flp.tile([P, HC], dt.float32, tag="tmpo")
                    nc.vector.tensor_scalar_mul(tmpo[:], outb[:], NEG_ACT)
                    nc.vector.tensor_tensor(
                        out=outbf[:], in0=outb[:], in1=tmpo[:], op=alu.max
                    )
                else:
                    nc.vector.tensor_tensor(
                        out=outbf[:], in0=d["outb"][:], in1=d["outb2"][:], op=alu.max
                    )
                d["outbf"] = outbf

            def flush_pool(b):
                d = pend.pop(b)
                nc.tensor.matmul(
                    pooled_ps[:],
                    lhsT=d["goh"][:],
                    rhs=d["outbf"][:],
                    start=(b == 0),
                    stop=(b == BLOCKS - 1),
                )

            # prologue: drep(0), drep(1), prep(0)
            dreps = {0: issue_drep(0), 1: issue_drep(1)}
            build_prep_dve(0, dreps.pop(0))
            build_prep_pe(0)
            for s in range(NSB):
                gwin = gwp.tile(
                    [P, NCHUNK, WIN_T, REC_F32], dt.float32, tag="gwin", bufs=3
                )
                for c in range(NCHUNK):
                    off = (s * NCHUNK + c) * WIN_MAIN
                    ix1 = ixp.tile([128, WIN_MAIN // 16], dt.int16, tag="ix1", bufs=8)
                    nc.sync.dma_start(
                        ix1[:], isrc_in[:, off // 16 : (off + WIN_MAIN) // 16]
                    )
                    nc.gpsimd.dma_gather(
                        out_ap=gwin[:, c, :, :],
                        in_ap=table[c * CHUNK_ROWS : (c + 1) * CHUNK_ROWS, :],
                        idxs_ap=ix1[:],
                        num_idxs=WIN_MAIN,
                        num_idxs_reg=WIN_MAIN,
                        elem_size=REC_F32,
                        single_packet=False,
                        queue_num=c % NQ,
                    )

                for bb in range(SB):
                    b = s * SB + bb
                    sl = slice(bb * t_bc, (bb + 1) * t_bc)
                    oh, _ohT, adst_ps = prep.pop(b)
                    pblk = edp.tile([P, T_BLK, 4], dt.float32, tag="pblk")
                    nc.vector.tensor_tensor(
                        out=pblk[:].rearrange("p (c t) h -> p c t h", c=NCHUNK),
                        in0=gwin[:, :, sl, 32:36],
                        in1=adst_ps[:].rearrange("p (c t) h -> p c t h", c=NCHUNK),
                        op=alu.add,
                    )
                    # exp(leaky_s(x)) == max(exp(x), exp(s*x)) for 0<s<1
                    pexp1 = edp.tile([P, T_BLK, 4], dt.float32, tag="pexp1")
                    nc.scalar.activation(pexp1[:], pblk[:], act.Exp)
                    pexp2 = edp.tile([P, T_BLK, 4], dt.float32, tag="pexp2")
                    nc.scalar.activation(pexp2[:], pblk[:], act.Exp, scale=NEG_ATT)
                    # DVE fillers while the scalar exps run
                    if b > 0:
                        flush_dve1(b - 1)
                    if b + 2 < BLOCKS:
                        dreps[b + 2] = issue_drep(b + 2)
                    if b + 1 < BLOCKS:
                        build_prep_dve(b + 1, dreps.pop(b + 1))
                    pbf = edp.tile([P, T_BLK, 4], dt.bfloat16, tag="pbf")
                    nc.vector.tensor_tensor(
                        out=pbf[:], in0=pexp1[:], in1=pexp2[:], op=alu.max
                    )

                    acc = psp.tile(
                        [P, HC + 4], dt.float32, space="PSUM", tag="acc", bufs=2
                    )
                    msg = edp.tile(
                        [P, T_BLK, HC + 4], dt.bfloat16, tag="msg", bufs=3
                    )
                    nc.vector.tensor_copy(msg[:, :, HC : HC + 4], pbf[:])
                    gb = gwin[:, :, sl, :].bitcast(dt.float8e4)
                    for c in range(NCHUNK):
                        slp = slice(c * t_bc, (c + 1) * t_bc)
                        nc.vector.tensor_tensor(
                            out=msg[:, slp, 0:HC].rearrange(
                                "p t (h c2) -> p t h c2", h=H
                            ),
                            in0=gb[:, c, :, 0:HC].rearrange(
                                "p t (h c2) -> p t h c2", h=H
                            ),
                            in1=pbf[:, slp, :]
                            .rearrange("p t h -> p t h ()")
                            .to_broadcast((P, t_bc, H, C)),
                            op=alu.mult,
                        )
                    for t_in_blk in range(T_BLK):
                        nc.tensor.matmul(
                            acc[:],
                            lhsT=oh[:, t_in_blk, :],
                            rhs=msg[:, t_in_blk, :],
                            start=(t_in_blk == 0),
                            stop=(t_in_blk == T_BLK - 1),
                        )
                    if b + 1 < BLOCKS:
                        build_prep_pe(b + 1)
                    goh = flp.tile([P, B], dt.bfloat16, tag="goh")
                    nc.sync.dma_start(goh[:], goh_in[b])
                    pend[b] = {"acc": acc, "goh": goh}
                    if b > 0:
                        flush_scalar(b - 1)
                        flush_dve2(b - 1)
                        flush_pool(b - 1)
            flush_dve1(BLOCKS - 1)
            flush_scalar(BLOCKS - 1)
            flush_dve2(BLOCKS - 1)
            flush_pool(BLOCKS - 1)

            # ---------- pooling + final linear ----------
            pooled_sb = nap.tile([B, HC], dt.float32, tag="poolsb", bufs=1)
            nc.vector.tensor_copy(pooled_sb[:], pooled_ps[:])
            nc.sync.dma_start(pool_in[:], pooled_sb[:])
            nc.gpsimd.collective_compute(
                "AllReduce",
                alu.add,
                replica_groups=[list(range(NCORES))],
                ins=[pool_in[:]],
                outs=[pool_out[:]],
            )
            pooled2 = nap.tile([B, HC], dt.float32, tag="pool2", bufs=1)
            nc.sync.dma_start(pooled2[:], pool_out[:])
            crecip = nap.tile([B, 1], dt.float32, tag="crecip", bufs=1)
            nc.sync.dma_start(crecip[:], crecip_in[:])
            nc.vector.tensor_scalar(
                out=pooled2[:],
                in0=pooled2[:],
                scalar1=crecip[:],
                scalar2=None,
                op0=alu.mult,
            )
            p2T_ps = psp.tile([HC, B], dt.float32, space="PSUM", tag="misc", bufs=1)
            nc.tensor.transpose(p2T_ps[:], pooled2[:], ident[:])
            p2T = nap.tile([HC, B], dt.float32, tag="p2T", bufs=1)
            nc.vector.tensor_copy(p2T[:], p2T_ps[:])
            linw_t = nap.tile([HC, NCLS], dt.float32, tag="linw", bufs=1)
            nc.sync.dma_start(linw_t[:], linw_in[:])
            fin_ps = psp.tile([B, NCLS], dt.float32, space="PSUM", tag="misc", bufs=1)
            nc.tensor.matmul(
                fin_ps[:], lhsT=p2T[:], rhs=linw_t[:], start=True, stop=True
            )
            fin_sb = nap.tile([B, NCLS], dt.float32, tag="finsb", bufs=1)
            nc.vector.tensor_copy(fin_sb[:], fin_ps[:])
            linb_t = nap.tile([B, NCLS], dt.float32, tag="linb", bufs=1)
            nc.sync.dma_start(linb_t[:], linb_in[:])
            nc.vector.tensor_add(fin_sb[:], fin_sb[:], linb_t[:])
            nc.sync.dma_start(out_fin[:], fin_sb[:])

    nc.compile()
    return nc


def _in_maps(hp):
    maps = []
    for k in range(NCORES):
        maps.append(
            {
                "xT_in": hp["xT_tiles"][k],
                "W_in": hp["W"],
                "A8_in": hp["A8"],
                "ident_in": hp["identity"],
                "iota_in": hp["iota_bf"],
                "iotac_in": hp["iota_col"],
                "bias_in": hp["bias_rep"],
                "dummy_in": hp["dummy_row"],
                "isrc_in": hp["idx_src_w"][k],
                "dstrel_in": hp["dstrel_w"][k],
                "drow_in": hp["dstrel_row"][k],
                "goh_in": hp["g_onehot"][k],
                "crecip_in": hp["counts_recip"],
                "linw_in": hp["linwT"],
                "linb_in": hp["linb_rep"],
            }
        )
    return maps


def kernel(x, edge_index, batch, batch_size, W, att_src, att_dst, bias, lin_w, lin_b):
    hp = _build_host_data(x, edge_index, batch, W, att_src, att_dst, bias, lin_w, lin_b)
    nc = _build_nc(hp)
    from concourse.bass_utils import run_bass_kernel_spmd

    res = run_bass_kernel_spmd(
        nc, _in_maps(hp), core_ids=list(range(NCORES)), trace=False
    )
    return np.asarray(res.results[0]["out_fin"], dtype=np.float32)
